# revision 14
# baseline (speedup 1.0000x reference)
"""Trainium2 Bass kernel for nn_AttentionOutput (complex causal leaky-relu attention).

Reference (B=4, N=4096, F=64), per batch:
    sr = (Qr@Kr^T - Qi@Ki^T)/sqrt(N); si = (Qr@Ki^T + Qi@Kr^T)/sqrt(N)
    wr = tril * leaky_relu(sr);        wi = tril * leaky_relu(si)
    out_r = (wr@Vr)@W_att^T + b;       out_i = (wi@Vi)@W_att^T + b

Distribution: 2 cores per batch.  Core parity h processes j-blocks J === h
(mod 2) for ALL 4096 query rows; causal work is then identical across cores
(slot I needs 2I+2 j-blocks), so a single SPMD program serves all 8 cores and
the host sums the two partial outputs per batch.

Host-side layout prep removes every on-device transpose:
  - scores contract over p = f*2+c (128 partitions, ONE matmul per component):
    sr = Qmodr . K^T where Qmodr = Q with odd columns negated, and
    si = Qmodi . K^T where Qmodi = Q with column pairs swapped; K stays plain.
    Both Q variants are fed pre-transposed [128, N].
  - V' = (1/64) V @ W_att^T folds the score scale and the output projection
    into the attention-value matmul (leaky_relu is positively homogeneous).
  - output is stored transposed ([128, N]: y_r^T on rows 0:64, y_i^T on
    64:128); the host untransposes, interleaves, adds bias, sums parities.

v2 structure (vs v1 baseline at 131us):
  - scores for r and i go into ONE 3D PSUM tile [128, 2, 512] (2 banks);
    the relu drain covers both components in ONE DVE/ACT op (fixed-cost
    amortized), alternating VectorE / ScalarE by a 1:2 pattern.
  - y_r^T and y_i^T share ONE [128, 512] PSUM bank: value matmuls are
    col-group packed (r -> partitions 0:64 via stationary cols 0:64,
    i -> partitions 64:128).  The PE runs such M=64 pairs concurrently in
    disjoint column groups, roughly halving value-matmul time.
  - diagonal blocks: the second diag j-block of each slot is fully masked
    for i-offsets < 256 on both core parities, so score matmuls, drains and
    value matmuls subrange to [256:512] there.
  - PSUM: 3 score bufs (6 banks) + 2 y bufs (2 banks) = 8 banks.

leaky_relu lowering (RELU_CORR): leaky(s) = 0.99*relu(s) + 0.01*s.  For
causally-full j-blocks the 0.01*s term telescopes into a per-slot constant
matmul: mcorr = 0.01*sum_J kp_J @ V'_J, precomputed on the host and
accumulated into the y PSUM bank.  Diagonal tiles compute u = mask*s
(VectorE, drains) and w = relu(u), feeding matmuls against 0.01*V' and
0.99*V'.

NOTE: ACT Lrelu reading PSUM hangs TRN2 (empirically) — never emit it.
"""

import numpy as np

import concourse.bacc as bacc
import concourse.tile as tile
from concourse import mybir
from concourse.bass_utils import run_bass_kernel_spmd

B, N, F = 4, 4096, 64
P = 128             # = 2*F: score contraction width / partition count
JB = 128            # j-block width
IBW = 512           # i-block (slot) width
NSLOT = N // IBW    # 8 slots
NJPAR = N // JB // 2  # 16 parity j-blocks per core
NEG = 0.01
SCALE = 1.0 / 64.0  # 1/sqrt(N)
NCORES = 8
DIAG1_OFF = 256     # both parities: 2nd diag block dead for i-offset < 256

_DT = mybir.dt.float32
MM_BF16 = True      # bf16 matmul inputs: half the DMA bytes, 4x fp32 PE rate
SIM_SAFE_LRELU = True  # kept for test.py compat (unused)
_CACHE: dict = {}


def _build_nc():
    nc = bacc.Bacc("TRN2", target_bir_lowering=False, num_devices=NCORES)
    dt = _DT
    mdt = mybir.dt.bfloat16 if MM_BF16 else _DT  # matmul input dtype
    qrT = nc.dram_tensor("qrT", [P, N], mdt, kind="ExternalInput")
    qiT = nc.dram_tensor("qiT", [P, N], mdt, kind="ExternalInput")
    kp = nc.dram_tensor("kp", [P, NJPAR * JB], mdt, kind="ExternalInput")
    # va = 0.99 * V' (relu term), vb = 0.01 * V' (raw term, diagonal only)
    var_ = nc.dram_tensor("var", [P, NJPAR * F], mdt, kind="ExternalInput")
    vai = nc.dram_tensor("vai", [P, NJPAR * F], mdt, kind="ExternalInput")
    vbr = nc.dram_tensor("vbr", [P, NJPAR * F], mdt, kind="ExternalInput")
    vbi = nc.dram_tensor("vbi", [P, NJPAR * F], mdt, kind="ExternalInput")
    # per-slot correction weights: 0.01 * sum_{full J} kp_J @ V'_J  [P, 64]
    mcr = nc.dram_tensor("mcr", [P, NSLOT * F], mdt, kind="ExternalInput")
    mci = nc.dram_tensor("mci", [P, NSLOT * F], mdt, kind="ExternalInput")
    # masks duplicated for both components: [2, JB, 2, IBW]
    dmask = nc.dram_tensor("dmask", [2, JB, 2, IBW], mdt, kind="ExternalInput")
    out = nc.dram_tensor("out", [P, N], mdt, kind="ExternalOutput")

    relu = mybir.ActivationFunctionType.Relu
    mul_op = mybir.AluOpType.mult
    max_op = mybir.AluOpType.max
    add_op = mybir.AluOpType.add

    with tile.TileContext(nc) as tc:
        with (
            tc.tile_pool(name="res", bufs=1) as res,
            tc.tile_pool(name="wp", bufs=3) as wp,
            tc.tile_pool(name="osb", bufs=2) as osb,
            tc.tile_pool(name="spsum", bufs=3, space="PSUM") as spsum,
            tc.tile_pool(name="ypsum", bufs=2, space="PSUM") as ypsum,
        ):
            # ---- input staging; order so slot 0's operands land first ----
            sb_qr = res.tile([P, N], mdt, tag="qr")
            sb_qi = res.tile([P, N], mdt, tag="qi")
            sb_k = res.tile([P, NJPAR * JB], mdt, tag="k")
            sb_m0 = res.tile([JB, 2, IBW], mdt, tag="m0")
            sb_m1 = res.tile([JB, 2, IBW], mdt, tag="m1")
            sb_var = res.tile([P, NJPAR * F], mdt, tag="var")
            sb_vai = res.tile([P, NJPAR * F], mdt, tag="vai")
            sb_vbr = res.tile([P, NJPAR * F], mdt, tag="vbr")
            sb_vbi = res.tile([P, NJPAR * F], mdt, tag="vbi")
            sb_mcr = res.tile([P, NSLOT * F], mdt, tag="mcr")
            sb_mci = res.tile([P, NSLOT * F], mdt, tag="mci")

            # q/k stream on the Sync queue; masks/values/mcorr on the GpSimd
            # queue.  Chunks are sized/ordered to land just before the slot
            # that first needs them (slot s: q chunk s, k cols < (2s+2)*128,
            # vb cols < (2s+2)*64 for diag, va cols < 2s*64, mcorr col s*64).
            nc.sync.dma_start(out=sb_qr[:, 0:512], in_=qrT[:, 0:512])
            nc.sync.dma_start(out=sb_k[:, 0:256], in_=kp[:, 0:256])
            nc.sync.dma_start(out=sb_qi[:, 0:512], in_=qiT[:, 0:512])
            nc.sync.dma_start(out=sb_k[:, 256:512], in_=kp[:, 256:512])
            nc.sync.dma_start(out=sb_qr[:, 512:1024], in_=qrT[:, 512:1024])
            nc.sync.dma_start(out=sb_qi[:, 512:1024], in_=qiT[:, 512:1024])
            nc.sync.dma_start(out=sb_k[:, 512:1024], in_=kp[:, 512:1024])
            for c in range(2, 8):
                sl = slice(c * 512, (c + 1) * 512)
                nc.sync.dma_start(out=sb_qr[:, sl], in_=qrT[:, sl])
                nc.sync.dma_start(out=sb_qi[:, sl], in_=qiT[:, sl])
                if c == 3:
                    nc.sync.dma_start(out=sb_k[:, 1024:2048],
                                      in_=kp[:, 1024:2048])
            nc.gpsimd.dma_start(out=sb_m0, in_=dmask[0])
            nc.gpsimd.dma_start(out=sb_m1, in_=dmask[1])
            nc.gpsimd.dma_start(out=sb_vbr[:, 0:256], in_=vbr[:, 0:256])
            nc.gpsimd.dma_start(out=sb_vbi[:, 0:256], in_=vbi[:, 0:256])
            nc.gpsimd.dma_start(out=sb_mcr[:, 0:192], in_=mcr[:, 0:192])
            nc.gpsimd.dma_start(out=sb_mci[:, 0:192], in_=mci[:, 0:192])
            nc.gpsimd.dma_start(out=sb_var[:, 0:256], in_=var_[:, 0:256])
            nc.gpsimd.dma_start(out=sb_vai[:, 0:256], in_=vai[:, 0:256])
            nc.gpsimd.dma_start(out=sb_vbr[:, 256:512], in_=vbr[:, 256:512])
            nc.gpsimd.dma_start(out=sb_vbi[:, 256:512], in_=vbi[:, 256:512])
            nc.gpsimd.dma_start(out=sb_var[:, 256:512], in_=var_[:, 256:512])
            nc.gpsimd.dma_start(out=sb_vai[:, 256:512], in_=vai[:, 256:512])
            nc.gpsimd.dma_start(out=sb_mcr[:, 192:512], in_=mcr[:, 192:512])
            nc.gpsimd.dma_start(out=sb_mci[:, 192:512], in_=mci[:, 192:512])
            nc.gpsimd.dma_start(out=sb_vbr[:, 512:1024], in_=vbr[:, 512:1024])
            nc.gpsimd.dma_start(out=sb_vbi[:, 512:1024], in_=vbi[:, 512:1024])
            nc.gpsimd.dma_start(out=sb_var[:, 512:1024], in_=var_[:, 512:1024])
            nc.gpsimd.dma_start(out=sb_vai[:, 512:1024], in_=vai[:, 512:1024])
            sb_masks = (sb_m0, sb_m1)

            # Warm the PE (HAM clock gate) with dummy matmuls while the first
            # DMAs are in flight, sized to span the DMA head so real matmuls
            # start at 2.4 GHz without being queued behind the warm-up.
            warm_sb = res.tile([P, F], mdt, tag="warm")
            nc.vector.memset(warm_sb[:], 0.0)
            warm_y = ypsum.tile([P, IBW], dt, tag="y")
            for _ in range(18):
                nc.tensor.matmul(warm_y[0:64, 0:64], warm_sb[:], warm_sb[:],
                                 start=True, stop=True)

            drain_ctr = 0  # full-tile relu drains: cycle V,S,S
            for s in range(NSLOT):
                cnt = 2 * s + 2
                isl = slice(s * IBW, (s + 1) * IBW)
                y = ypsum.tile([P, IBW], dt, tag="y")

                # 1) diagonal blocks' scores + drain chains FIRST: their
                #    VectorE latency (mask-mult, then w2 = max(100u, u) =
                #    u + 99*relu(u)) hides under the full blocks' matmuls.
                w2s = []
                for k in range(2):
                    p = cnt - 2 + k
                    ksl = slice(p * JB, (p + 1) * JB)
                    o = DIAG1_OFF if k == 1 else 0
                    qsl = slice(s * IBW + o, (s + 1) * IBW)
                    s2 = spsum.tile([P, 2, IBW], dt, tag="s")
                    nc.tensor.matmul(s2[:, 0:1, o:], sb_k[:, ksl],
                                     sb_qr[:, qsl], start=True, stop=True)
                    nc.tensor.matmul(s2[:, 1:2, o:], sb_k[:, ksl],
                                     sb_qi[:, qsl], start=True, stop=True)
                    mk = sb_masks[k]
                    u = wp.tile([P, 2, IBW], mdt, tag="u", bufs=2)
                    nc.vector.tensor_tensor(out=u[:, :, o:], in0=s2[:, :, o:],
                                            in1=mk[:, :, o:], op=mul_op)
                    w2 = wp.tile([P, 2, IBW], mdt, tag="w2", bufs=4)
                    nc.vector.scalar_tensor_tensor(
                        out=w2[:, :, o:], in0=u[:, :, o:], scalar=100.0,
                        in1=u[:, :, o:], op0=mul_op, op1=max_op)
                    w2s.append((w2, o, p))

                # 2) correction pair opens the y accumulation group (s>0):
                #    y += (0.01 * sum_full kp_J @ V'_J)^T @ q
                if s > 0:
                    msl = slice(s * F, (s + 1) * F)
                    nc.tensor.matmul(y[0:64, :], sb_mcr[:, msl],
                                     sb_qr[:, isl], start=True, stop=False)
                    nc.tensor.matmul(y[64:128, :], sb_mci[:, msl],
                                     sb_qi[:, isl], start=True, stop=False)

                # 3) full blocks: w = relu(s); 0.01*s is in mcorr
                for p in range(cnt - 2):
                    ksl = slice(p * JB, (p + 1) * JB)
                    vsl = slice(p * F, (p + 1) * F)
                    qsl = isl
                    s2 = spsum.tile([P, 2, IBW], dt, tag="s")
                    nc.tensor.matmul(s2[:, 0:1, :], sb_k[:, ksl],
                                     sb_qr[:, qsl], start=True, stop=True)
                    nc.tensor.matmul(s2[:, 1:2, :], sb_k[:, ksl],
                                     sb_qi[:, qsl], start=True, stop=True)
                    w = wp.tile([P, 2, IBW], mdt, tag="w")
                    if drain_ctr % 3 == 0:
                        nc.vector.tensor_scalar_max(w[:], s2[:], 0.0)
                    else:
                        nc.scalar.activation(w[:], s2[:], relu)
                    drain_ctr += 1
                    nc.tensor.matmul(y[0:64, :], sb_var[:, vsl], w[:, 0:1, :],
                                     start=False, stop=False)
                    nc.tensor.matmul(y[64:128, :], sb_vai[:, vsl],
                                     w[:, 1:2, :], start=False, stop=False)

                # 4) diagonal vb pairs close the slot (drains long done):
                #    vb @ w2 = 0.01*V'*u + 0.99*V'*relu(u)
                for k, (w2, o, p) in enumerate(w2s):
                    vsl = slice(p * F, (p + 1) * F)
                    first = s == 0 and k == 0
                    last = k == 1
                    nc.tensor.matmul(y[0:64, o:], sb_vbr[:, vsl],
                                     w2[:, 0:1, o:], start=first, stop=last)
                    nc.tensor.matmul(y[64:128, o:], sb_vbi[:, vsl],
                                     w2[:, 1:2, o:], start=first, stop=last)
                # tail: accumulator to SBUF bf16 (alternate V/S), DMA out;
                # the last slot's DMA is split so its exposed tail halves
                y_sb = osb.tile([P, IBW], mdt, tag="ysb")
                if s % 2 == 0:
                    nc.vector.tensor_copy(y_sb[:], y[:])
                else:
                    nc.scalar.copy(y_sb[:], y[:])
                if s == NSLOT - 1:
                    nc.sync.dma_start(out=out[0:64, isl], in_=y_sb[0:64, :])
                    nc.gpsimd.dma_start(out=out[64:128, isl], in_=y_sb[64:128, :])
                else:
                    nc.sync.dma_start(out=out[:, isl], in_=y_sb[:])
    nc.compile()
    return nc


def _prep_inputs(Q, K, V, W_att, b_att):
    """Host-side re-layout: per-core in_maps for run_bass_kernel_spmd."""
    Q = np.asarray(Q, dtype=np.float32)
    K = np.asarray(K, dtype=np.float32)
    V = np.asarray(V, dtype=np.float32)
    W_att = np.asarray(W_att, dtype=np.float32)

    Qf = Q.reshape(B, N, P)          # [b, i, f*2+c]
    Kf = K.reshape(B, N, P)
    Vpr = SCALE * (V[..., 0] @ W_att.T)   # [B, N, F]
    Vpi = SCALE * (V[..., 1] @ W_att.T)

    # causal masks for a slot's last two parity j-blocks, per core parity h:
    # diagonal sub-block d = 2k+h of the slot's group of 4; duplicated along
    # a component axis -> [2, JB, 2, IBW]
    jj = np.arange(JB)[:, None]
    ii = np.arange(IBW)[None, :]
    masks = {}
    for h in (0, 1):
        mk = np.stack([(ii >= jj + JB * (2 * k + h)).astype(np.float32)
                       for k in range(2)])            # [2, JB, IBW]
        masks[h] = np.repeat(mk[:, :, None, :], 2, axis=2)  # [2, JB, 2, IBW]

    if MM_BF16:
        import ml_dtypes
        cvt = lambda a: np.ascontiguousarray(a).astype(ml_dtypes.bfloat16)
    else:
        cvt = lambda a: np.ascontiguousarray(a, dtype=np.float32)

    in_maps = []
    for c in range(NCORES):
        b, h = divmod(c, 2)
        Qmodr = Qf[b].copy()
        Qmodr[:, 1::2] *= -1.0
        Qmodi = np.empty_like(Qf[b])
        Qmodi[:, 0::2] = Qf[b][:, 1::2]
        Qmodi[:, 1::2] = Qf[b][:, 0::2]
        # parity-packed K: [P, NJPAR*JB], position pp holds block J = 2*pp+h
        kp3 = Kf[b].reshape(N // JB, JB, P)[h::2]          # [16, j, p]
        kp = kp3.transpose(2, 0, 1).reshape(P, -1)         # [p, pp*JB+j]
        vr3 = Vpr[b].reshape(N // JB, JB, F)[h::2]         # [16, j, f]
        vi3 = Vpi[b].reshape(N // JB, JB, F)[h::2]
        vpr = vr3.transpose(1, 0, 2).reshape(JB, -1)       # [j, pp*F+f]
        vpi = vi3.transpose(1, 0, 2).reshape(JB, -1)
        # per-slot correction: 0.01 * sum over FULL blocks (pos < cnt-2 = 2s)
        prod_r = np.einsum('bjp,bjf->bpf', kp3, vr3)       # [16, p, f]
        prod_i = np.einsum('bjp,bjf->bpf', kp3, vi3)
        pre_r = np.concatenate(
            [np.zeros((1, P, F), np.float32), np.cumsum(prod_r, axis=0)])
        pre_i = np.concatenate(
            [np.zeros((1, P, F), np.float32), np.cumsum(prod_i, axis=0)])
        mcr = np.concatenate([NEG * pre_r[2 * s] for s in range(NSLOT)], axis=1)
        mci = np.concatenate([NEG * pre_i[2 * s] for s in range(NSLOT)], axis=1)
        in_maps.append({
            "qrT": cvt(Qmodr.T),
            "qiT": cvt(Qmodi.T),
            "kp": cvt(kp),
            "var": cvt((1.0 - NEG) * vpr),
            "vai": cvt((1.0 - NEG) * vpi),
            "vbr": cvt(NEG * vpr),
            "vbi": cvt(NEG * vpi),
            "mcr": cvt(mcr),
            "mci": cvt(mci),
            "dmask": cvt(masks[h]),
        })
    return in_maps


def _gather(results, b_att):
    b_att = np.asarray(b_att, dtype=np.float32)
    out = np.empty((B, N, F, 2), dtype=np.float32)
    for b in range(B):
        y = (np.asarray(results[2 * b]["out"], dtype=np.float32)
             + np.asarray(results[2 * b + 1]["out"], dtype=np.float32))
        out[b, :, :, 0] = y[0:64].T + b_att[None, :]
        out[b, :, :, 1] = y[64:128].T + b_att[None, :]
    return out


def kernel(Q, K, V, W_att, b_att):
    if "nc" not in _CACHE:
        _CACHE["nc"] = _build_nc()
    nc = _CACHE["nc"]
    in_maps = _prep_inputs(Q, K, V, W_att, b_att)
    res = run_bass_kernel_spmd(nc, in_maps, core_ids=list(range(NCORES)))
    return _gather(res.results, b_att)


# revision 18
# speedup vs baseline: 1.0220x; 1.0220x over previous
"""Trainium2 Bass kernel for nn_AttentionOutput (complex causal leaky-relu attention).

Reference (B=4, N=4096, F=64), per batch:
    sr = (Qr@Kr^T - Qi@Ki^T)/sqrt(N); si = (Qr@Ki^T + Qi@Kr^T)/sqrt(N)
    wr = tril * leaky_relu(sr);        wi = tril * leaky_relu(si)
    out_r = (wr@Vr)@W_att^T + b;       out_i = (wi@Vi)@W_att^T + b

Distribution: 2 cores per batch.  Core parity h processes j-blocks J === h
(mod 2) for ALL 4096 query rows; causal work is then identical across cores
(slot I needs 2I+2 j-blocks), so a single SPMD program serves all 8 cores and
the host sums the two partial outputs per batch.

Host-side layout prep removes every on-device transpose:
  - scores contract over p = f*2+c (128 partitions, ONE matmul per component):
    sr = Qmodr . K^T where Qmodr = Q with odd columns negated, and
    si = Qmodi . K^T where Qmodi = Q with column pairs swapped; K stays plain.
    Both Q variants are fed pre-transposed [128, N].
  - V' = (1/64) V @ W_att^T folds the score scale and the output projection
    into the attention-value matmul (leaky_relu is positively homogeneous).
  - output is stored transposed ([128, N]: y_r^T on rows 0:64, y_i^T on
    64:128); the host untransposes, interleaves, adds bias, sums parities.

v2 structure (vs v1 baseline at 131us):
  - scores for r and i go into ONE 3D PSUM tile [128, 2, 512] (2 banks);
    the relu drain covers both components in ONE DVE/ACT op (fixed-cost
    amortized), alternating VectorE / ScalarE by a 1:2 pattern.
  - y_r^T and y_i^T share ONE [128, 512] PSUM bank: value matmuls are
    col-group packed (r -> partitions 0:64 via stationary cols 0:64,
    i -> partitions 64:128).  The PE runs such M=64 pairs concurrently in
    disjoint column groups, roughly halving value-matmul time.
  - diagonal blocks: the second diag j-block of each slot is fully masked
    for i-offsets < 256 on both core parities, so score matmuls, drains and
    value matmuls subrange to [256:512] there.
  - PSUM: 3 score bufs (6 banks) + 2 y bufs (2 banks) = 8 banks.

leaky_relu lowering (RELU_CORR): leaky(s) = 0.99*relu(s) + 0.01*s.  For
causally-full j-blocks the 0.01*s term telescopes into a per-slot constant
matmul: mcorr = 0.01*sum_J kp_J @ V'_J, precomputed on the host and
accumulated into the y PSUM bank.  Diagonal tiles compute u = mask*s
(VectorE, drains) and w = relu(u), feeding matmuls against 0.01*V' and
0.99*V'.

NOTE: ACT Lrelu reading PSUM hangs TRN2 (empirically) — never emit it.
"""

import numpy as np

import concourse.bacc as bacc
import concourse.tile as tile
from concourse import mybir
from concourse.bass_utils import run_bass_kernel_spmd

B, N, F = 4, 4096, 64
P = 128             # = 2*F: score contraction width / partition count
JB = 128            # j-block width
IBW = 512           # i-block (slot) width
NSLOT = N // IBW    # 8 slots
NJPAR = N // JB // 2  # 16 parity j-blocks per core
NEG = 0.01
SCALE = 1.0 / 64.0  # 1/sqrt(N)
NCORES = 8
DIAG1_OFF = 256     # both parities: 2nd diag block dead for i-offset < 256

_DT = mybir.dt.float32
MM_BF16 = True      # bf16 matmul inputs: half the DMA bytes, 4x fp32 PE rate
SIM_SAFE_LRELU = True  # kept for test.py compat (unused)
_CACHE: dict = {}


def _build_nc():
    nc = bacc.Bacc("TRN2", target_bir_lowering=False, num_devices=NCORES)
    dt = _DT
    mdt = mybir.dt.bfloat16 if MM_BF16 else _DT  # matmul input dtype
    qrT = nc.dram_tensor("qrT", [P, N], mdt, kind="ExternalInput")
    qiT = nc.dram_tensor("qiT", [P, N], mdt, kind="ExternalInput")
    kp = nc.dram_tensor("kp", [P, NJPAR * JB], mdt, kind="ExternalInput")
    # va = 0.99 * V' (relu term), vb = 0.01 * V' (raw term, diagonal only)
    var_ = nc.dram_tensor("var", [P, NJPAR * F], mdt, kind="ExternalInput")
    vai = nc.dram_tensor("vai", [P, NJPAR * F], mdt, kind="ExternalInput")
    vbr = nc.dram_tensor("vbr", [P, NJPAR * F], mdt, kind="ExternalInput")
    vbi = nc.dram_tensor("vbi", [P, NJPAR * F], mdt, kind="ExternalInput")
    # per-slot correction weights: 0.01 * sum_{full J} kp_J @ V'_J  [P, 64]
    mcr = nc.dram_tensor("mcr", [P, NSLOT * F], mdt, kind="ExternalInput")
    mci = nc.dram_tensor("mci", [P, NSLOT * F], mdt, kind="ExternalInput")
    # one mask per diag block k (broadcast over components on device)
    dmask = nc.dram_tensor("dmask", [JB, 2, IBW], mdt, kind="ExternalInput")
    out = nc.dram_tensor("out", [P, N], mdt, kind="ExternalOutput")

    relu = mybir.ActivationFunctionType.Relu
    mul_op = mybir.AluOpType.mult
    max_op = mybir.AluOpType.max
    add_op = mybir.AluOpType.add

    with tile.TileContext(nc) as tc:
        with (
            tc.tile_pool(name="res", bufs=1) as res,
            tc.tile_pool(name="wp", bufs=3) as wp,
            tc.tile_pool(name="osb", bufs=2) as osb,
            tc.tile_pool(name="spsum", bufs=3, space="PSUM") as spsum,
            tc.tile_pool(name="ypsum", bufs=2, space="PSUM") as ypsum,
        ):
            # ---- input staging; order so slot 0's operands land first ----
            sb_qr = res.tile([P, N], mdt, tag="qr")
            sb_qi = res.tile([P, N], mdt, tag="qi")
            sb_k = res.tile([P, NJPAR * JB], mdt, tag="k")
            sb_m = res.tile([JB, 2, IBW], mdt, tag="m")
            sb_var = res.tile([P, NJPAR * F], mdt, tag="var")
            sb_vai = res.tile([P, NJPAR * F], mdt, tag="vai")
            sb_vbr = res.tile([P, NJPAR * F], mdt, tag="vbr")
            sb_vbi = res.tile([P, NJPAR * F], mdt, tag="vbi")
            sb_mcr = res.tile([P, NSLOT * F], mdt, tag="mcr")
            sb_mci = res.tile([P, NSLOT * F], mdt, tag="mci")

            # All input DMAs on the Sync queue, ordered by first use under the
            # diag-first slot schedule (slot s diag needs k cols
            # [2s*128:(2s+2)*128] and vb cols [2s*64:(2s+2)*64] at its START,
            # full blocks need va cols < 2s*64, mcorr col s*64 early).
            nc.sync.dma_start(out=sb_qr[:, 0:512], in_=qrT[:, 0:512])
            nc.sync.dma_start(out=sb_k[:, 0:256], in_=kp[:, 0:256])
            nc.sync.dma_start(out=sb_qi[:, 0:512], in_=qiT[:, 0:512])
            nc.sync.dma_start(out=sb_m, in_=dmask[:])
            nc.sync.dma_start(out=sb_vbr[:, 0:512], in_=vbr[:, 0:512])
            nc.sync.dma_start(out=sb_vbi[:, 0:512], in_=vbi[:, 0:512])
            nc.sync.dma_start(out=sb_k[:, 256:512], in_=kp[:, 256:512])
            nc.sync.dma_start(out=sb_qr[:, 512:1024], in_=qrT[:, 512:1024])
            nc.sync.dma_start(out=sb_qi[:, 512:1024], in_=qiT[:, 512:1024])
            nc.sync.dma_start(out=sb_var[:, 0:512], in_=var_[:, 0:512])
            nc.sync.dma_start(out=sb_vai[:, 0:512], in_=vai[:, 0:512])
            nc.sync.dma_start(out=sb_mcr, in_=mcr[:])
            nc.sync.dma_start(out=sb_mci, in_=mci[:])
            nc.sync.dma_start(out=sb_k[:, 512:1024], in_=kp[:, 512:1024])
            for c in range(2, 8):
                sl = slice(c * 512, (c + 1) * 512)
                nc.sync.dma_start(out=sb_qr[:, sl], in_=qrT[:, sl])
                nc.sync.dma_start(out=sb_qi[:, sl], in_=qiT[:, sl])
                if c == 3:
                    nc.sync.dma_start(out=sb_k[:, 1024:2048],
                                      in_=kp[:, 1024:2048])
                if c == 4:
                    nc.sync.dma_start(out=sb_vbr[:, 512:1024],
                                      in_=vbr[:, 512:1024])
                    nc.sync.dma_start(out=sb_vbi[:, 512:1024],
                                      in_=vbi[:, 512:1024])
                    nc.sync.dma_start(out=sb_var[:, 512:1024],
                                      in_=var_[:, 512:1024])
                    nc.sync.dma_start(out=sb_vai[:, 512:1024],
                                      in_=vai[:, 512:1024])
            sb_masks = tuple(
                sb_m[:, k:k + 1, :].broadcast_to([JB, 2, IBW])
                for k in range(2))

            # Warm the PE (HAM clock gate) with dummy matmuls while the first
            # DMAs are in flight, sized to span the DMA head so real matmuls
            # start at 2.4 GHz without being queued behind the warm-up.
            warm_sb = res.tile([P, F], mdt, tag="warm")
            nc.vector.memset(warm_sb[:], 0.0)
            warm_y = ypsum.tile([P, IBW], dt, tag="y")
            for _ in range(18):
                nc.tensor.matmul(warm_y[0:64, 0:64], warm_sb[:], warm_sb[:],
                                 start=True, stop=True)

            drain_ctr = 0  # full-tile relu drains: cycle V,S,S
            for s in range(NSLOT):
                cnt = 2 * s + 2
                isl = slice(s * IBW, (s + 1) * IBW)
                y = ypsum.tile([P, IBW], dt, tag="y")

                # 1) diagonal blocks' scores + drain chains FIRST: their
                #    VectorE latency (mask-mult, then w2 = max(100u, u) =
                #    u + 99*relu(u)) hides under the full blocks' matmuls.
                w2s = []
                for k in range(2):
                    p = cnt - 2 + k
                    ksl = slice(p * JB, (p + 1) * JB)
                    o = DIAG1_OFF if k == 1 else 0
                    qsl = slice(s * IBW + o, (s + 1) * IBW)
                    s2 = spsum.tile([P, 2, IBW], dt, tag="s")
                    nc.tensor.matmul(s2[:, 0:1, o:], sb_k[:, ksl],
                                     sb_qr[:, qsl], start=True, stop=True)
                    nc.tensor.matmul(s2[:, 1:2, o:], sb_k[:, ksl],
                                     sb_qi[:, qsl], start=True, stop=True)
                    mk = sb_masks[k]
                    u = wp.tile([P, 2, IBW], mdt, tag="u", bufs=2)
                    nc.vector.tensor_tensor(out=u[:, :, o:], in0=s2[:, :, o:],
                                            in1=mk[:, :, o:], op=mul_op)
                    w2 = wp.tile([P, 2, IBW], mdt, tag="w2", bufs=4)
                    nc.vector.scalar_tensor_tensor(
                        out=w2[:, :, o:], in0=u[:, :, o:], scalar=100.0,
                        in1=u[:, :, o:], op0=mul_op, op1=max_op)
                    w2s.append((w2, o, p))

                # 2) correction pair opens the y accumulation group (s>0):
                #    y += (0.01 * sum_full kp_J @ V'_J)^T @ q
                if s > 0:
                    msl = slice(s * F, (s + 1) * F)
                    nc.tensor.matmul(y[0:64, :], sb_mcr[:, msl],
                                     sb_qr[:, isl], start=True, stop=False)
                    nc.tensor.matmul(y[64:128, :], sb_mci[:, msl],
                                     sb_qi[:, isl], start=True, stop=False)

                # 3) full blocks: w = relu(s); 0.01*s is in mcorr
                for p in range(cnt - 2):
                    ksl = slice(p * JB, (p + 1) * JB)
                    vsl = slice(p * F, (p + 1) * F)
                    qsl = isl
                    s2 = spsum.tile([P, 2, IBW], dt, tag="s")
                    nc.tensor.matmul(s2[:, 0:1, :], sb_k[:, ksl],
                                     sb_qr[:, qsl], start=True, stop=True)
                    nc.tensor.matmul(s2[:, 1:2, :], sb_k[:, ksl],
                                     sb_qi[:, qsl], start=True, stop=True)
                    w = wp.tile([P, 2, IBW], mdt, tag="w")
                    if drain_ctr % 4 == 0:
                        nc.vector.tensor_scalar_max(w[:], s2[:], 0.0)
                    else:
                        nc.scalar.activation(w[:], s2[:], relu)
                    drain_ctr += 1
                    nc.tensor.matmul(y[0:64, :], sb_var[:, vsl], w[:, 0:1, :],
                                     start=False, stop=False)
                    nc.tensor.matmul(y[64:128, :], sb_vai[:, vsl],
                                     w[:, 1:2, :], start=False, stop=False)

                # 4) diagonal vb pairs close the slot (drains long done):
                #    vb @ w2 = 0.01*V'*u + 0.99*V'*relu(u)
                for k, (w2, o, p) in enumerate(w2s):
                    vsl = slice(p * F, (p + 1) * F)
                    first = s == 0 and k == 0
                    last = k == 1
                    nc.tensor.matmul(y[0:64, o:], sb_vbr[:, vsl],
                                     w2[:, 0:1, o:], start=first, stop=last)
                    nc.tensor.matmul(y[64:128, o:], sb_vbi[:, vsl],
                                     w2[:, 1:2, o:], start=first, stop=last)
                # tail: accumulator to SBUF bf16 (alternate V/S), DMA out;
                # the last slot's DMA is split so its exposed tail halves
                y_sb = osb.tile([P, IBW], mdt, tag="ysb")
                if s % 2 == 0:
                    nc.vector.tensor_copy(y_sb[:], y[:])
                else:
                    nc.scalar.copy(y_sb[:], y[:])
                if s == NSLOT - 1:
                    nc.sync.dma_start(out=out[0:64, isl], in_=y_sb[0:64, :])
                    nc.gpsimd.dma_start(out=out[64:128, isl], in_=y_sb[64:128, :])
                else:
                    nc.sync.dma_start(out=out[:, isl], in_=y_sb[:])
    nc.compile()
    return nc


def _prep_inputs(Q, K, V, W_att, b_att):
    """Host-side re-layout: per-core in_maps for run_bass_kernel_spmd."""
    Q = np.asarray(Q, dtype=np.float32)
    K = np.asarray(K, dtype=np.float32)
    V = np.asarray(V, dtype=np.float32)
    W_att = np.asarray(W_att, dtype=np.float32)

    Qf = Q.reshape(B, N, P)          # [b, i, f*2+c]
    Kf = K.reshape(B, N, P)
    Vpr = SCALE * (V[..., 0] @ W_att.T)   # [B, N, F]
    Vpi = SCALE * (V[..., 1] @ W_att.T)

    # causal masks for a slot's last two parity j-blocks, per core parity h:
    # diagonal sub-block d = 2k+h of the slot's group of 4; duplicated along
    # a component axis -> [2, JB, 2, IBW]
    jj = np.arange(JB)[:, None]
    ii = np.arange(IBW)[None, :]
    masks = {}
    for h in (0, 1):
        masks[h] = np.stack([(ii >= jj + JB * (2 * k + h)).astype(np.float32)
                             for k in range(2)], axis=1)   # [JB, 2, IBW]

    if MM_BF16:
        import ml_dtypes
        cvt = lambda a: np.ascontiguousarray(a).astype(ml_dtypes.bfloat16)
    else:
        cvt = lambda a: np.ascontiguousarray(a, dtype=np.float32)

    in_maps = []
    for c in range(NCORES):
        b, h = divmod(c, 2)
        Qmodr = Qf[b].copy()
        Qmodr[:, 1::2] *= -1.0
        Qmodi = np.empty_like(Qf[b])
        Qmodi[:, 0::2] = Qf[b][:, 1::2]
        Qmodi[:, 1::2] = Qf[b][:, 0::2]
        # parity-packed K: [P, NJPAR*JB], position pp holds block J = 2*pp+h
        kp3 = Kf[b].reshape(N // JB, JB, P)[h::2]          # [16, j, p]
        kp = kp3.transpose(2, 0, 1).reshape(P, -1)         # [p, pp*JB+j]
        vr3 = Vpr[b].reshape(N // JB, JB, F)[h::2]         # [16, j, f]
        vi3 = Vpi[b].reshape(N // JB, JB, F)[h::2]
        vpr = vr3.transpose(1, 0, 2).reshape(JB, -1)       # [j, pp*F+f]
        vpi = vi3.transpose(1, 0, 2).reshape(JB, -1)
        # per-slot correction: 0.01 * sum over FULL blocks (pos < cnt-2 = 2s)
        prod_r = np.einsum('bjp,bjf->bpf', kp3, vr3)       # [16, p, f]
        prod_i = np.einsum('bjp,bjf->bpf', kp3, vi3)
        pre_r = np.concatenate(
            [np.zeros((1, P, F), np.float32), np.cumsum(prod_r, axis=0)])
        pre_i = np.concatenate(
            [np.zeros((1, P, F), np.float32), np.cumsum(prod_i, axis=0)])
        mcr = np.concatenate([NEG * pre_r[2 * s] for s in range(NSLOT)], axis=1)
        mci = np.concatenate([NEG * pre_i[2 * s] for s in range(NSLOT)], axis=1)
        in_maps.append({
            "qrT": cvt(Qmodr.T),
            "qiT": cvt(Qmodi.T),
            "kp": cvt(kp),
            "var": cvt((1.0 - NEG) * vpr),
            "vai": cvt((1.0 - NEG) * vpi),
            "vbr": cvt(NEG * vpr),
            "vbi": cvt(NEG * vpi),
            "mcr": cvt(mcr),
            "mci": cvt(mci),
            "dmask": cvt(masks[h]),
        })
    return in_maps


def _gather(results, b_att):
    b_att = np.asarray(b_att, dtype=np.float32)
    out = np.empty((B, N, F, 2), dtype=np.float32)
    for b in range(B):
        y = (np.asarray(results[2 * b]["out"], dtype=np.float32)
             + np.asarray(results[2 * b + 1]["out"], dtype=np.float32))
        out[b, :, :, 0] = y[0:64].T + b_att[None, :]
        out[b, :, :, 1] = y[64:128].T + b_att[None, :]
    return out


def kernel(Q, K, V, W_att, b_att):
    if "nc" not in _CACHE:
        _CACHE["nc"] = _build_nc()
    nc = _CACHE["nc"]
    in_maps = _prep_inputs(Q, K, V, W_att, b_att)
    res = run_bass_kernel_spmd(nc, in_maps, core_ids=list(range(NCORES)))
    return _gather(res.results, b_att)


# revision 19
# speedup vs baseline: 1.0339x; 1.0116x over previous
"""Trainium2 Bass kernel for nn_AttentionOutput (complex causal leaky-relu attention).

Reference (B=4, N=4096, F=64), per batch:
    sr = (Qr@Kr^T - Qi@Ki^T)/sqrt(N); si = (Qr@Ki^T + Qi@Kr^T)/sqrt(N)
    wr = tril * leaky_relu(sr);        wi = tril * leaky_relu(si)
    out_r = (wr@Vr)@W_att^T + b;       out_i = (wi@Vi)@W_att^T + b

Distribution: 2 cores per batch.  Core parity h processes j-blocks J === h
(mod 2) for ALL 4096 query rows; causal work is then identical across cores
(slot I needs 2I+2 j-blocks), so a single SPMD program serves all 8 cores and
the host sums the two partial outputs per batch.

Host-side layout prep removes every on-device transpose:
  - scores contract over p = f*2+c (128 partitions, ONE matmul per component):
    sr = Qmodr . K^T where Qmodr = Q with odd columns negated, and
    si = Qmodi . K^T where Qmodi = Q with column pairs swapped; K stays plain.
    Both Q variants are fed pre-transposed [128, N].
  - V' = (1/64) V @ W_att^T folds the score scale and the output projection
    into the attention-value matmul (leaky_relu is positively homogeneous).
  - output is stored transposed ([128, N]: y_r^T on rows 0:64, y_i^T on
    64:128); the host untransposes, interleaves, adds bias, sums parities.

v2 structure (vs v1 baseline at 131us):
  - scores for r and i go into ONE 3D PSUM tile [128, 2, 512] (2 banks);
    the relu drain covers both components in ONE DVE/ACT op (fixed-cost
    amortized), alternating VectorE / ScalarE by a 1:2 pattern.
  - y_r^T and y_i^T share ONE [128, 512] PSUM bank: value matmuls are
    col-group packed (r -> partitions 0:64 via stationary cols 0:64,
    i -> partitions 64:128).  The PE runs such M=64 pairs concurrently in
    disjoint column groups, roughly halving value-matmul time.
  - diagonal blocks: the second diag j-block of each slot is fully masked
    for i-offsets < 256 on both core parities, so score matmuls, drains and
    value matmuls subrange to [256:512] there.
  - PSUM: 3 score bufs (6 banks) + 2 y bufs (2 banks) = 8 banks.

leaky_relu lowering (RELU_CORR): leaky(s) = 0.99*relu(s) + 0.01*s.  For
causally-full j-blocks the 0.01*s term telescopes into a per-slot constant
matmul: mcorr = 0.01*sum_J kp_J @ V'_J, precomputed on the host and
accumulated into the y PSUM bank.  Diagonal tiles compute u = mask*s
(VectorE, drains) and w = relu(u), feeding matmuls against 0.01*V' and
0.99*V'.

NOTE: ACT Lrelu reading PSUM hangs TRN2 (empirically) — never emit it.
"""

import numpy as np

import concourse.bacc as bacc
import concourse.tile as tile
from concourse import mybir
from concourse.bass_utils import run_bass_kernel_spmd

B, N, F = 4, 4096, 64
P = 128             # = 2*F: score contraction width / partition count
JB = 128            # j-block width
IBW = 512           # i-block (slot) width
NSLOT = N // IBW    # 8 slots
NJPAR = N // JB // 2  # 16 parity j-blocks per core
NEG = 0.01
SCALE = 1.0 / 64.0  # 1/sqrt(N)
NCORES = 8
DIAG1_OFF = 256     # both parities: 2nd diag block dead for i-offset < 256

_DT = mybir.dt.float32
MM_BF16 = True      # bf16 matmul inputs: half the DMA bytes, 4x fp32 PE rate
SIM_SAFE_LRELU = True  # kept for test.py compat (unused)
_CACHE: dict = {}


def _build_nc():
    nc = bacc.Bacc("TRN2", target_bir_lowering=False, num_devices=NCORES)
    dt = _DT
    mdt = mybir.dt.bfloat16 if MM_BF16 else _DT  # matmul input dtype
    qrT = nc.dram_tensor("qrT", [P, N], mdt, kind="ExternalInput")
    qiT = nc.dram_tensor("qiT", [P, N], mdt, kind="ExternalInput")
    # fused copy of q (dim1 = component) for the late chunks: one DMA
    # descriptor covers both components
    q2f = nc.dram_tensor("q2f", [P, 2, N], mdt, kind="ExternalInput")
    kp = nc.dram_tensor("kp", [P, NJPAR * JB], mdt, kind="ExternalInput")
    # va = 0.99 * V' (relu term), vb = 0.01 * V' (raw term, diagonal only);
    # dim1 = component
    va2 = nc.dram_tensor("va2", [P, 2, NJPAR * F], mdt, kind="ExternalInput")
    vb2 = nc.dram_tensor("vb2", [P, 2, NJPAR * F], mdt, kind="ExternalInput")
    # per-slot correction weights: 0.01 * sum_{full J} kp_J @ V'_J  [P, 64]
    mc2 = nc.dram_tensor("mc2", [P, 2, NSLOT * F], mdt, kind="ExternalInput")
    # one mask per diag block k (broadcast over components on device)
    dmask = nc.dram_tensor("dmask", [JB, 2, IBW], mdt, kind="ExternalInput")
    out = nc.dram_tensor("out", [P, N], mdt, kind="ExternalOutput")

    relu = mybir.ActivationFunctionType.Relu
    mul_op = mybir.AluOpType.mult
    max_op = mybir.AluOpType.max
    add_op = mybir.AluOpType.add

    with tile.TileContext(nc) as tc:
        with (
            tc.tile_pool(name="res", bufs=1) as res,
            tc.tile_pool(name="wp", bufs=3) as wp,
            tc.tile_pool(name="osb", bufs=2) as osb,
            tc.tile_pool(name="spsum", bufs=3, space="PSUM") as spsum,
            tc.tile_pool(name="ypsum", bufs=2, space="PSUM") as ypsum,
        ):
            # ---- input staging; order so slot 0's operands land first ----
            sb_q = res.tile([P, 2, N], mdt, tag="q")
            sb_k = res.tile([P, NJPAR * JB], mdt, tag="k")
            sb_m = res.tile([JB, 2, IBW], mdt, tag="m")
            sb_va = res.tile([P, 2, NJPAR * F], mdt, tag="va")
            sb_vb = res.tile([P, 2, NJPAR * F], mdt, tag="vb")
            sb_mc = res.tile([P, 2, NSLOT * F], mdt, tag="mc")

            # All input DMAs on the Sync queue (out-DMAs go to GpSimd),
            # ordered by first use under the diag-first slot schedule.
            # Early q chunks are split per component for latency; late ones
            # and all v/mcorr tensors are component-fused (1 descriptor).
            nc.sync.dma_start(out=sb_q[:, 0:1, 0:512], in_=qrT[:, 0:512])
            nc.sync.dma_start(out=sb_k[:, 0:256], in_=kp[:, 0:256])
            nc.sync.dma_start(out=sb_q[:, 1:2, 0:512], in_=qiT[:, 0:512])
            nc.sync.dma_start(out=sb_m[:, 0:1, :], in_=dmask[:, 0:1, :])
            nc.sync.dma_start(out=sb_vb[:, :, 0:256], in_=vb2[:, :, 0:256])
            nc.sync.dma_start(out=sb_m[:, 1:2, :], in_=dmask[:, 1:2, :])
            nc.sync.dma_start(out=sb_k[:, 256:512], in_=kp[:, 256:512])
            nc.sync.dma_start(out=sb_q[:, 0:1, 512:1024], in_=qrT[:, 512:1024])
            nc.sync.dma_start(out=sb_q[:, 1:2, 512:1024], in_=qiT[:, 512:1024])
            nc.sync.dma_start(out=sb_va[:, :, 0:256], in_=va2[:, :, 0:256])
            nc.sync.dma_start(out=sb_mc[:, :, 0:192], in_=mc2[:, :, 0:192])
            nc.sync.dma_start(out=sb_k[:, 512:1024], in_=kp[:, 512:1024])
            nc.sync.dma_start(out=sb_q[:, 0:1, 1024:1536], in_=qrT[:, 1024:1536])
            nc.sync.dma_start(out=sb_q[:, 1:2, 1024:1536], in_=qiT[:, 1024:1536])
            nc.sync.dma_start(out=sb_vb[:, :, 256:512], in_=vb2[:, :, 256:512])
            nc.sync.dma_start(out=sb_q[:, 0:1, 1536:2048], in_=qrT[:, 1536:2048])
            nc.sync.dma_start(out=sb_q[:, 1:2, 1536:2048], in_=qiT[:, 1536:2048])
            nc.sync.dma_start(out=sb_va[:, :, 256:512], in_=va2[:, :, 256:512])
            nc.sync.dma_start(out=sb_mc[:, :, 192:512], in_=mc2[:, :, 192:512])
            nc.sync.dma_start(out=sb_k[:, 1024:1536], in_=kp[:, 1024:1536])
            nc.sync.dma_start(out=sb_q[:, :, 2048:2560], in_=q2f[:, :, 2048:2560])
            nc.sync.dma_start(out=sb_vb[:, :, 512:1024], in_=vb2[:, :, 512:1024])
            nc.sync.dma_start(out=sb_q[:, :, 2560:3072], in_=q2f[:, :, 2560:3072])
            nc.sync.dma_start(out=sb_va[:, :, 512:1024], in_=va2[:, :, 512:1024])
            nc.sync.dma_start(out=sb_k[:, 1536:2048], in_=kp[:, 1536:2048])
            nc.sync.dma_start(out=sb_q[:, :, 3072:3584], in_=q2f[:, :, 3072:3584])
            nc.sync.dma_start(out=sb_q[:, :, 3584:4096], in_=q2f[:, :, 3584:4096])
            sb_masks = tuple(
                sb_m[:, k:k + 1, :].broadcast_to([JB, 2, IBW])
                for k in range(2))

            # Warm the PE (HAM clock gate) with dummy matmuls while the first
            # DMAs are in flight, sized to span the DMA head so real matmuls
            # start at 2.4 GHz without being queued behind the warm-up.
            warm_sb = res.tile([P, F], mdt, tag="warm")
            nc.vector.memset(warm_sb[:], 0.0)
            warm_y = ypsum.tile([P, IBW], dt, tag="y")
            for _ in range(18):
                nc.tensor.matmul(warm_y[0:64, 0:64], warm_sb[:], warm_sb[:],
                                 start=True, stop=True)

            drain_ctr = 0  # full-tile relu drains: cycle V,S,S
            for s in range(NSLOT):
                cnt = 2 * s + 2
                isl = slice(s * IBW, (s + 1) * IBW)
                y = ypsum.tile([P, IBW], dt, tag="y")

                # 1) diagonal blocks' scores + drain chains FIRST: their
                #    VectorE latency (mask-mult, then w2 = max(100u, u) =
                #    u + 99*relu(u)) hides under the full blocks' matmuls.
                w2s = []
                for k in range(2):
                    p = cnt - 2 + k
                    ksl = slice(p * JB, (p + 1) * JB)
                    o = DIAG1_OFF if k == 1 else 0
                    qsl = slice(s * IBW + o, (s + 1) * IBW)
                    s2 = spsum.tile([P, 2, IBW], dt, tag="s")
                    nc.tensor.matmul(s2[:, 0:1, o:], sb_k[:, ksl],
                                     sb_q[:, 0:1, qsl], start=True, stop=True)
                    nc.tensor.matmul(s2[:, 1:2, o:], sb_k[:, ksl],
                                     sb_q[:, 1:2, qsl], start=True, stop=True)
                    mk = sb_masks[k]
                    u = wp.tile([P, 2, IBW], mdt, tag="u", bufs=2)
                    nc.vector.tensor_tensor(out=u[:, :, o:], in0=s2[:, :, o:],
                                            in1=mk[:, :, o:], op=mul_op)
                    w2 = wp.tile([P, 2, IBW], mdt, tag="w2", bufs=4)
                    nc.vector.scalar_tensor_tensor(
                        out=w2[:, :, o:], in0=u[:, :, o:], scalar=100.0,
                        in1=u[:, :, o:], op0=mul_op, op1=max_op)
                    w2s.append((w2, o, p))

                # 2) correction pair opens the y accumulation group (s>0):
                #    y += (0.01 * sum_full kp_J @ V'_J)^T @ q
                if s > 0:
                    msl = slice(s * F, (s + 1) * F)
                    nc.tensor.matmul(y[0:64, :], sb_mc[:, 0:1, msl],
                                     sb_q[:, 0:1, isl], start=True, stop=False)
                    nc.tensor.matmul(y[64:128, :], sb_mc[:, 1:2, msl],
                                     sb_q[:, 1:2, isl], start=True, stop=False)

                # 3) full blocks: w = relu(s); 0.01*s is in mcorr
                for p in range(cnt - 2):
                    ksl = slice(p * JB, (p + 1) * JB)
                    vsl = slice(p * F, (p + 1) * F)
                    qsl = isl
                    s2 = spsum.tile([P, 2, IBW], dt, tag="s")
                    nc.tensor.matmul(s2[:, 0:1, :], sb_k[:, ksl],
                                     sb_q[:, 0:1, qsl], start=True, stop=True)
                    nc.tensor.matmul(s2[:, 1:2, :], sb_k[:, ksl],
                                     sb_q[:, 1:2, qsl], start=True, stop=True)
                    w = wp.tile([P, 2, IBW], mdt, tag="w")
                    if drain_ctr % 4 == 0:
                        nc.vector.tensor_scalar_max(w[:], s2[:], 0.0)
                    else:
                        nc.scalar.activation(w[:], s2[:], relu)
                    drain_ctr += 1
                    nc.tensor.matmul(y[0:64, :], sb_va[:, 0:1, vsl],
                                     w[:, 0:1, :], start=False, stop=False)
                    nc.tensor.matmul(y[64:128, :], sb_va[:, 1:2, vsl],
                                     w[:, 1:2, :], start=False, stop=False)

                # 4) diagonal vb pairs close the slot (drains long done):
                #    vb @ w2 = 0.01*V'*u + 0.99*V'*relu(u)
                for k, (w2, o, p) in enumerate(w2s):
                    vsl = slice(p * F, (p + 1) * F)
                    first = s == 0 and k == 0
                    last = k == 1
                    nc.tensor.matmul(y[0:64, o:], sb_vb[:, 0:1, vsl],
                                     w2[:, 0:1, o:], start=first, stop=last)
                    nc.tensor.matmul(y[64:128, o:], sb_vb[:, 1:2, vsl],
                                     w2[:, 1:2, o:], start=first, stop=last)
                # tail: accumulator to SBUF bf16 (alternate V/S), DMA out;
                # the last slot's DMA is split so its exposed tail halves
                y_sb = osb.tile([P, IBW], mdt, tag="ysb")
                if s % 2 == 0:
                    nc.vector.tensor_copy(y_sb[:], y[:])
                else:
                    nc.scalar.copy(y_sb[:], y[:])
                if s == NSLOT - 1:
                    nc.gpsimd.dma_start(out=out[0:64, isl], in_=y_sb[0:64, :])
                    nc.sync.dma_start(out=out[64:128, isl], in_=y_sb[64:128, :])
                else:
                    nc.gpsimd.dma_start(out=out[:, isl], in_=y_sb[:])
    nc.compile()
    return nc


def _prep_inputs(Q, K, V, W_att, b_att):
    """Host-side re-layout: per-core in_maps for run_bass_kernel_spmd."""
    Q = np.asarray(Q, dtype=np.float32)
    K = np.asarray(K, dtype=np.float32)
    V = np.asarray(V, dtype=np.float32)
    W_att = np.asarray(W_att, dtype=np.float32)

    Qf = Q.reshape(B, N, P)          # [b, i, f*2+c]
    Kf = K.reshape(B, N, P)
    Vpr = SCALE * (V[..., 0] @ W_att.T)   # [B, N, F]
    Vpi = SCALE * (V[..., 1] @ W_att.T)

    # causal masks for a slot's last two parity j-blocks, per core parity h:
    # diagonal sub-block d = 2k+h of the slot's group of 4; duplicated along
    # a component axis -> [2, JB, 2, IBW]
    jj = np.arange(JB)[:, None]
    ii = np.arange(IBW)[None, :]
    masks = {}
    for h in (0, 1):
        masks[h] = np.stack([(ii >= jj + JB * (2 * k + h)).astype(np.float32)
                             for k in range(2)], axis=1)   # [JB, 2, IBW]

    if MM_BF16:
        import ml_dtypes
        cvt = lambda a: np.ascontiguousarray(a).astype(ml_dtypes.bfloat16)
    else:
        cvt = lambda a: np.ascontiguousarray(a, dtype=np.float32)

    in_maps = []
    for c in range(NCORES):
        b, h = divmod(c, 2)
        Qmodr = Qf[b].copy()
        Qmodr[:, 1::2] *= -1.0
        Qmodi = np.empty_like(Qf[b])
        Qmodi[:, 0::2] = Qf[b][:, 1::2]
        Qmodi[:, 1::2] = Qf[b][:, 0::2]
        # parity-packed K: [P, NJPAR*JB], position pp holds block J = 2*pp+h
        kp3 = Kf[b].reshape(N // JB, JB, P)[h::2]          # [16, j, p]
        kp = kp3.transpose(2, 0, 1).reshape(P, -1)         # [p, pp*JB+j]
        vr3 = Vpr[b].reshape(N // JB, JB, F)[h::2]         # [16, j, f]
        vi3 = Vpi[b].reshape(N // JB, JB, F)[h::2]
        vpr = vr3.transpose(1, 0, 2).reshape(JB, -1)       # [j, pp*F+f]
        vpi = vi3.transpose(1, 0, 2).reshape(JB, -1)
        # per-slot correction: 0.01 * sum over FULL blocks (pos < cnt-2 = 2s)
        prod_r = np.einsum('bjp,bjf->bpf', kp3, vr3)       # [16, p, f]
        prod_i = np.einsum('bjp,bjf->bpf', kp3, vi3)
        pre_r = np.concatenate(
            [np.zeros((1, P, F), np.float32), np.cumsum(prod_r, axis=0)])
        pre_i = np.concatenate(
            [np.zeros((1, P, F), np.float32), np.cumsum(prod_i, axis=0)])
        mcr = np.concatenate([NEG * pre_r[2 * s] for s in range(NSLOT)], axis=1)
        mci = np.concatenate([NEG * pre_i[2 * s] for s in range(NSLOT)], axis=1)
        qrt, qit = Qmodr.T, Qmodi.T
        in_maps.append({
            "qrT": cvt(qrt),
            "qiT": cvt(qit),
            "q2f": cvt(np.stack([qrt, qit], axis=1)),
            "kp": cvt(kp),
            "va2": cvt((1.0 - NEG) * np.stack([vpr, vpi], axis=1)),
            "vb2": cvt(NEG * np.stack([vpr, vpi], axis=1)),
            "mc2": cvt(np.stack([mcr, mci], axis=1)),
            "dmask": cvt(masks[h]),
        })
    return in_maps


def _gather(results, b_att):
    b_att = np.asarray(b_att, dtype=np.float32)
    out = np.empty((B, N, F, 2), dtype=np.float32)
    for b in range(B):
        y = (np.asarray(results[2 * b]["out"], dtype=np.float32)
             + np.asarray(results[2 * b + 1]["out"], dtype=np.float32))
        out[b, :, :, 0] = y[0:64].T + b_att[None, :]
        out[b, :, :, 1] = y[64:128].T + b_att[None, :]
    return out


def kernel(Q, K, V, W_att, b_att):
    if "nc" not in _CACHE:
        _CACHE["nc"] = _build_nc()
    nc = _CACHE["nc"]
    in_maps = _prep_inputs(Q, K, V, W_att, b_att)
    res = run_bass_kernel_spmd(nc, in_maps, core_ids=list(range(NCORES)))
    return _gather(res.results, b_att)


# revision 20
# speedup vs baseline: 1.1117x; 1.0753x over previous
"""Trainium2 Bass kernel for nn_AttentionOutput (complex causal leaky-relu attention).

Reference (B=4, N=4096, F=64), per batch:
    sr = (Qr@Kr^T - Qi@Ki^T)/sqrt(N); si = (Qr@Ki^T + Qi@Kr^T)/sqrt(N)
    wr = tril * leaky_relu(sr);        wi = tril * leaky_relu(si)
    out_r = (wr@Vr)@W_att^T + b;       out_i = (wi@Vi)@W_att^T + b

Distribution: 2 cores per batch.  Core parity h processes j-blocks J === h
(mod 2) for ALL 4096 query rows; causal work is then identical across cores
(slot I needs 2I+2 j-blocks), so a single SPMD program serves all 8 cores and
the host sums the two partial outputs per batch.

Host-side layout prep removes every on-device transpose:
  - scores contract over p = f*2+c (128 partitions, ONE matmul per component):
    sr = Qmodr . K^T where Qmodr = Q with odd columns negated, and
    si = Qmodi . K^T where Qmodi = Q with column pairs swapped; K stays plain.
    Both Q variants are fed pre-transposed [128, N].
  - V' = (1/64) V @ W_att^T folds the score scale and the output projection
    into the attention-value matmul (leaky_relu is positively homogeneous).
  - output is stored transposed ([128, N]: y_r^T on rows 0:64, y_i^T on
    64:128); the host untransposes, interleaves, adds bias, sums parities.

v2 structure (vs v1 baseline at 131us):
  - scores for r and i go into ONE 3D PSUM tile [128, 2, 512] (2 banks);
    the relu drain covers both components in ONE DVE/ACT op (fixed-cost
    amortized), alternating VectorE / ScalarE by a 1:2 pattern.
  - y_r^T and y_i^T share ONE [128, 512] PSUM bank: value matmuls are
    col-group packed (r -> partitions 0:64 via stationary cols 0:64,
    i -> partitions 64:128).  The PE runs such M=64 pairs concurrently in
    disjoint column groups, roughly halving value-matmul time.
  - diagonal blocks: the second diag j-block of each slot is fully masked
    for i-offsets < 256 on both core parities, so score matmuls, drains and
    value matmuls subrange to [256:512] there.
  - PSUM: 3 score bufs (6 banks) + 2 y bufs (2 banks) = 8 banks.

leaky_relu lowering (RELU_CORR): leaky(s) = 0.99*relu(s) + 0.01*s.  For
causally-full j-blocks the 0.01*s term telescopes into a per-slot constant
matmul: mcorr = 0.01*sum_J kp_J @ V'_J, precomputed on the host and
accumulated into the y PSUM bank.  Diagonal tiles compute u = mask*s
(VectorE, drains) and w = relu(u), feeding matmuls against 0.01*V' and
0.99*V'.

NOTE: ACT Lrelu reading PSUM hangs TRN2 (empirically) — never emit it.
"""

import numpy as np

import concourse.bacc as bacc
import concourse.tile as tile
from concourse import mybir
from concourse.bass_utils import run_bass_kernel_spmd

B, N, F = 4, 4096, 64
P = 128             # = 2*F: score contraction width / partition count
JB = 128            # j-block width
IBW = 512           # i-block (slot) width
NSLOT = N // IBW    # 8 slots
NJPAR = N // JB // 2  # 16 parity j-blocks per core
NEG = 0.01
SCALE = 1.0 / 64.0  # 1/sqrt(N)
NCORES = 8
DIAG1_OFF = 256     # both parities: 2nd diag block dead for i-offset < 256

_DT = mybir.dt.float32
MM_BF16 = True      # bf16 matmul inputs: half the DMA bytes, 4x fp32 PE rate
SIM_SAFE_LRELU = True  # kept for test.py compat (unused)
_CACHE: dict = {}


def _build_nc():
    nc = bacc.Bacc("TRN2", target_bir_lowering=False, num_devices=NCORES)
    dt = _DT
    mdt = mybir.dt.bfloat16 if MM_BF16 else _DT  # matmul input dtype
    qrT = nc.dram_tensor("qrT", [P, N], mdt, kind="ExternalInput")
    qiT = nc.dram_tensor("qiT", [P, N], mdt, kind="ExternalInput")
    # fused copy of q (dim1 = component) for the late chunks: one DMA
    # descriptor covers both components
    q2f = nc.dram_tensor("q2f", [P, 2, N], mdt, kind="ExternalInput")
    kp = nc.dram_tensor("kp", [P, NJPAR * JB], mdt, kind="ExternalInput")
    # va = 0.99 * V' (relu term), vb = 0.01 * V' (raw term, diagonal only);
    # dim1 = component
    va2 = nc.dram_tensor("va2", [P, 2, NJPAR * F], mdt, kind="ExternalInput")
    vb2 = nc.dram_tensor("vb2", [P, 2, NJPAR * F], mdt, kind="ExternalInput")
    # per-slot correction weights: 0.01 * sum_{full J} kp_J @ V'_J  [P, 64]
    mc2 = nc.dram_tensor("mc2", [P, 2, NSLOT * F], mdt, kind="ExternalInput")
    # one mask per diag block k (broadcast over components on device)
    dmask = nc.dram_tensor("dmask", [JB, 2, IBW], mdt, kind="ExternalInput")
    out = nc.dram_tensor("out", [P, N], mdt, kind="ExternalOutput")

    relu = mybir.ActivationFunctionType.Relu
    mul_op = mybir.AluOpType.mult
    max_op = mybir.AluOpType.max
    add_op = mybir.AluOpType.add

    with tile.TileContext(nc) as tc:
        with (
            tc.tile_pool(name="res", bufs=1) as res,
            tc.tile_pool(name="wp", bufs=3) as wp,
            tc.tile_pool(name="osb", bufs=2) as osb,
            tc.tile_pool(name="spsum", bufs=3, space="PSUM") as spsum,
            tc.tile_pool(name="ypsum", bufs=2, space="PSUM") as ypsum,
        ):
            # ---- input staging; order so slot 0's operands land first ----
            sb_q = res.tile([P, 2, N], mdt, tag="q")
            sb_k = res.tile([P, NJPAR * JB], mdt, tag="k")
            sb_m = res.tile([JB, 2, IBW], mdt, tag="m")
            sb_va = res.tile([P, 2, NJPAR * F], mdt, tag="va")
            sb_vb = res.tile([P, 2, NJPAR * F], mdt, tag="vb")
            sb_mc = res.tile([P, 2, NSLOT * F], mdt, tag="mc")

            # All input DMAs on the Sync queue (out-DMAs go to GpSimd),
            # ordered by first use under the diag-first slot schedule.
            # Early q chunks are split per component for latency; late ones
            # and all v/mcorr tensors are component-fused (1 descriptor).
            nc.sync.dma_start(out=sb_q[:, 0:1, 0:512], in_=qrT[:, 0:512])
            nc.sync.dma_start(out=sb_k[:, 0:256], in_=kp[:, 0:256])
            nc.sync.dma_start(out=sb_q[:, 1:2, 0:512], in_=qiT[:, 0:512])
            nc.sync.dma_start(out=sb_m[:, 0:1, :], in_=dmask[:, 0:1, :])
            nc.sync.dma_start(out=sb_vb[:, :, 0:256], in_=vb2[:, :, 0:256])
            nc.sync.dma_start(out=sb_m[:, 1:2, :], in_=dmask[:, 1:2, :])
            nc.sync.dma_start(out=sb_k[:, 256:512], in_=kp[:, 256:512])
            nc.sync.dma_start(out=sb_q[:, 0:1, 512:1024], in_=qrT[:, 512:1024])
            nc.sync.dma_start(out=sb_q[:, 1:2, 512:1024], in_=qiT[:, 512:1024])
            nc.sync.dma_start(out=sb_va[:, :, 0:256], in_=va2[:, :, 0:256])
            nc.sync.dma_start(out=sb_mc[:, :, 0:192], in_=mc2[:, :, 0:192])
            nc.sync.dma_start(out=sb_k[:, 512:1024], in_=kp[:, 512:1024])
            nc.sync.dma_start(out=sb_q[:, 0:1, 1024:1536], in_=qrT[:, 1024:1536])
            nc.sync.dma_start(out=sb_q[:, 1:2, 1024:1536], in_=qiT[:, 1024:1536])
            nc.sync.dma_start(out=sb_vb[:, :, 256:512], in_=vb2[:, :, 256:512])
            nc.sync.dma_start(out=sb_q[:, 0:1, 1536:2048], in_=qrT[:, 1536:2048])
            nc.sync.dma_start(out=sb_q[:, 1:2, 1536:2048], in_=qiT[:, 1536:2048])
            nc.sync.dma_start(out=sb_va[:, :, 256:512], in_=va2[:, :, 256:512])
            nc.sync.dma_start(out=sb_mc[:, :, 192:512], in_=mc2[:, :, 192:512])
            nc.sync.dma_start(out=sb_k[:, 1024:1536], in_=kp[:, 1024:1536])
            nc.sync.dma_start(out=sb_q[:, :, 2048:2560], in_=q2f[:, :, 2048:2560])
            nc.sync.dma_start(out=sb_vb[:, :, 512:1024], in_=vb2[:, :, 512:1024])
            nc.sync.dma_start(out=sb_q[:, :, 2560:3072], in_=q2f[:, :, 2560:3072])
            nc.sync.dma_start(out=sb_va[:, :, 512:1024], in_=va2[:, :, 512:1024])
            nc.sync.dma_start(out=sb_k[:, 1536:2048], in_=kp[:, 1536:2048])
            nc.sync.dma_start(out=sb_q[:, :, 3072:3584], in_=q2f[:, :, 3072:3584])
            nc.sync.dma_start(out=sb_q[:, :, 3584:4096], in_=q2f[:, :, 3584:4096])
            sb_masks = tuple(
                sb_m[:, k:k + 1, :].broadcast_to([JB, 2, IBW])
                for k in range(2))

            # Warm the PE (HAM clock gate) with dummy matmuls while the first
            # DMAs are in flight, sized to span the DMA head so real matmuls
            # start at 2.4 GHz without being queued behind the warm-up.
            warm_sb = res.tile([P, F], mdt, tag="warm")
            nc.vector.memset(warm_sb[:], 0.0)
            warm_y = ypsum.tile([P, IBW], dt, tag="y")
            for _ in range(18):
                nc.tensor.matmul(warm_y[0:64, 0:64], warm_sb[:], warm_sb[:],
                                 start=True, stop=True)

            drain_ctr = 0  # full-tile relu drains: cycle V,S,S,S
            for s in range(NSLOT):
                cnt = 2 * s + 2
                nfull = cnt - 2
                isl = slice(s * IBW, (s + 1) * IBW)
                y = ypsum.tile([P, IBW], dt, tag="y")

                def sc_pair(p, o=0):
                    """Score matmul pair for j-block p into a fresh PSUM tile."""
                    ksl = slice(p * JB, (p + 1) * JB)
                    qsl = slice(s * IBW + o, (s + 1) * IBW)
                    s2 = spsum.tile([P, 2, IBW], dt, tag="s", name="s2")
                    nc.tensor.matmul(s2[:, 0:1, o:], sb_k[:, ksl],
                                     sb_q[:, 0:1, qsl], start=True, stop=True)
                    nc.tensor.matmul(s2[:, 1:2, o:], sb_k[:, ksl],
                                     sb_q[:, 1:2, qsl], start=True, stop=True)
                    return s2

                def val_pair(p, w, start):
                    vsl = slice(p * F, (p + 1) * F)
                    nc.tensor.matmul(y[0:64, :], sb_va[:, 0:1, vsl],
                                     w[:, 0:1, :], start=start, stop=False)
                    nc.tensor.matmul(y[64:128, :], sb_va[:, 1:2, vsl],
                                     w[:, 1:2, :], start=start, stop=False)

                # Full blocks, software-pipelined: the value pair for block p
                # is emitted after block p+1's scores so its drain has cover
                # (the tensor queue is strict FIFO).
                pend = None  # (p, w) awaiting its value pair
                for p in range(nfull):
                    s2 = sc_pair(p)
                    w = wp.tile([P, 2, IBW], mdt, tag="w")
                    if drain_ctr % 4 == 0:
                        nc.vector.tensor_scalar_max(w[:], s2[:], 0.0)
                    else:
                        nc.scalar.activation(w[:], s2[:], relu)
                    drain_ctr += 1
                    if pend is not None:
                        val_pair(pend[0], pend[1], pend[0] == 0)
                    pend = (p, w)

                # Diagonal blocks: u = mask*s (drains), w2 = max(100u, u) =
                # u + 99*relu(u), so vb @ w2 = 0.01*V'*u + 0.99*V'*relu(u).
                w2s = []
                for k in range(2):
                    p = nfull + k
                    o = DIAG1_OFF if k == 1 else 0
                    s2 = sc_pair(p, o)
                    mk = sb_masks[k]
                    u = wp.tile([P, 2, IBW], mdt, tag="u", bufs=2)
                    nc.vector.tensor_tensor(out=u[:, :, o:], in0=s2[:, :, o:],
                                            in1=mk[:, :, o:], op=mul_op)
                    w2 = wp.tile([P, 2, IBW], mdt, tag="w2", bufs=4)
                    nc.vector.scalar_tensor_tensor(
                        out=w2[:, :, o:], in0=u[:, :, o:], scalar=100.0,
                        in1=u[:, :, o:], op0=mul_op, op1=max_op)
                    w2s.append((w2, o, p))
                    if pend is not None:
                        val_pair(pend[0], pend[1], pend[0] == 0)
                        pend = None

                # Correction pair (no drain dependency) covers the diag
                # chains' latency: y += (0.01 * sum_full kp_J @ V'_J)^T @ q
                if s > 0:
                    msl = slice(s * F, (s + 1) * F)
                    nc.tensor.matmul(y[0:64, :], sb_mc[:, 0:1, msl],
                                     sb_q[:, 0:1, isl], start=False, stop=False)
                    nc.tensor.matmul(y[64:128, :], sb_mc[:, 1:2, msl],
                                     sb_q[:, 1:2, isl], start=False, stop=False)

                # Diagonal vb pairs close the slot.
                for k, (w2, o, p) in enumerate(w2s):
                    vsl = slice(p * F, (p + 1) * F)
                    first = s == 0 and k == 0
                    last = k == 1
                    nc.tensor.matmul(y[0:64, o:], sb_vb[:, 0:1, vsl],
                                     w2[:, 0:1, o:], start=first, stop=last)
                    nc.tensor.matmul(y[64:128, o:], sb_vb[:, 1:2, vsl],
                                     w2[:, 1:2, o:], start=first, stop=last)

                # tail: accumulator to SBUF bf16 (alternate V/S), DMA out;
                # the last slot's DMA is split so its exposed tail halves
                y_sb = osb.tile([P, IBW], mdt, tag="ysb")
                if s % 2 == 0:
                    nc.vector.tensor_copy(y_sb[:], y[:])
                else:
                    nc.scalar.copy(y_sb[:], y[:])
                if s == NSLOT - 1:
                    nc.gpsimd.dma_start(out=out[0:64, isl], in_=y_sb[0:64, :])
                    nc.sync.dma_start(out=out[64:128, isl], in_=y_sb[64:128, :])
                else:
                    nc.gpsimd.dma_start(out=out[:, isl], in_=y_sb[:])
    nc.compile()
    return nc


def _prep_inputs(Q, K, V, W_att, b_att):
    """Host-side re-layout: per-core in_maps for run_bass_kernel_spmd."""
    Q = np.asarray(Q, dtype=np.float32)
    K = np.asarray(K, dtype=np.float32)
    V = np.asarray(V, dtype=np.float32)
    W_att = np.asarray(W_att, dtype=np.float32)

    Qf = Q.reshape(B, N, P)          # [b, i, f*2+c]
    Kf = K.reshape(B, N, P)
    Vpr = SCALE * (V[..., 0] @ W_att.T)   # [B, N, F]
    Vpi = SCALE * (V[..., 1] @ W_att.T)

    # causal masks for a slot's last two parity j-blocks, per core parity h:
    # diagonal sub-block d = 2k+h of the slot's group of 4; duplicated along
    # a component axis -> [2, JB, 2, IBW]
    jj = np.arange(JB)[:, None]
    ii = np.arange(IBW)[None, :]
    masks = {}
    for h in (0, 1):
        masks[h] = np.stack([(ii >= jj + JB * (2 * k + h)).astype(np.float32)
                             for k in range(2)], axis=1)   # [JB, 2, IBW]

    if MM_BF16:
        import ml_dtypes
        cvt = lambda a: np.ascontiguousarray(a).astype(ml_dtypes.bfloat16)
    else:
        cvt = lambda a: np.ascontiguousarray(a, dtype=np.float32)

    in_maps = []
    for c in range(NCORES):
        b, h = divmod(c, 2)
        Qmodr = Qf[b].copy()
        Qmodr[:, 1::2] *= -1.0
        Qmodi = np.empty_like(Qf[b])
        Qmodi[:, 0::2] = Qf[b][:, 1::2]
        Qmodi[:, 1::2] = Qf[b][:, 0::2]
        # parity-packed K: [P, NJPAR*JB], position pp holds block J = 2*pp+h
        kp3 = Kf[b].reshape(N // JB, JB, P)[h::2]          # [16, j, p]
        kp = kp3.transpose(2, 0, 1).reshape(P, -1)         # [p, pp*JB+j]
        vr3 = Vpr[b].reshape(N // JB, JB, F)[h::2]         # [16, j, f]
        vi3 = Vpi[b].reshape(N // JB, JB, F)[h::2]
        vpr = vr3.transpose(1, 0, 2).reshape(JB, -1)       # [j, pp*F+f]
        vpi = vi3.transpose(1, 0, 2).reshape(JB, -1)
        # per-slot correction: 0.01 * sum over FULL blocks (pos < cnt-2 = 2s)
        prod_r = np.einsum('bjp,bjf->bpf', kp3, vr3)       # [16, p, f]
        prod_i = np.einsum('bjp,bjf->bpf', kp3, vi3)
        pre_r = np.concatenate(
            [np.zeros((1, P, F), np.float32), np.cumsum(prod_r, axis=0)])
        pre_i = np.concatenate(
            [np.zeros((1, P, F), np.float32), np.cumsum(prod_i, axis=0)])
        mcr = np.concatenate([NEG * pre_r[2 * s] for s in range(NSLOT)], axis=1)
        mci = np.concatenate([NEG * pre_i[2 * s] for s in range(NSLOT)], axis=1)
        qrt, qit = Qmodr.T, Qmodi.T
        in_maps.append({
            "qrT": cvt(qrt),
            "qiT": cvt(qit),
            "q2f": cvt(np.stack([qrt, qit], axis=1)),
            "kp": cvt(kp),
            "va2": cvt((1.0 - NEG) * np.stack([vpr, vpi], axis=1)),
            "vb2": cvt(NEG * np.stack([vpr, vpi], axis=1)),
            "mc2": cvt(np.stack([mcr, mci], axis=1)),
            "dmask": cvt(masks[h]),
        })
    return in_maps


def _gather(results, b_att):
    b_att = np.asarray(b_att, dtype=np.float32)
    out = np.empty((B, N, F, 2), dtype=np.float32)
    for b in range(B):
        y = (np.asarray(results[2 * b]["out"], dtype=np.float32)
             + np.asarray(results[2 * b + 1]["out"], dtype=np.float32))
        out[b, :, :, 0] = y[0:64].T + b_att[None, :]
        out[b, :, :, 1] = y[64:128].T + b_att[None, :]
    return out


def kernel(Q, K, V, W_att, b_att):
    if "nc" not in _CACHE:
        _CACHE["nc"] = _build_nc()
    nc = _CACHE["nc"]
    in_maps = _prep_inputs(Q, K, V, W_att, b_att)
    res = run_bass_kernel_spmd(nc, in_maps, core_ids=list(range(NCORES)))
    return _gather(res.results, b_att)


# revision 21
# speedup vs baseline: 1.1150x; 1.0029x over previous
"""Trainium2 Bass kernel for nn_AttentionOutput (complex causal leaky-relu attention).

Reference (B=4, N=4096, F=64), per batch:
    sr = (Qr@Kr^T - Qi@Ki^T)/sqrt(N); si = (Qr@Ki^T + Qi@Kr^T)/sqrt(N)
    wr = tril * leaky_relu(sr);        wi = tril * leaky_relu(si)
    out_r = (wr@Vr)@W_att^T + b;       out_i = (wi@Vi)@W_att^T + b

Distribution: 2 cores per batch.  Core parity h processes j-blocks J === h
(mod 2) for ALL 4096 query rows; causal work is then identical across cores
(slot I needs 2I+2 j-blocks), so a single SPMD program serves all 8 cores and
the host sums the two partial outputs per batch.

Host-side layout prep removes every on-device transpose:
  - scores contract over p = f*2+c (128 partitions, ONE matmul per component):
    sr = Qmodr . K^T where Qmodr = Q with odd columns negated, and
    si = Qmodi . K^T where Qmodi = Q with column pairs swapped; K stays plain.
    Both Q variants are fed pre-transposed [128, N].
  - V' = (1/64) V @ W_att^T folds the score scale and the output projection
    into the attention-value matmul (leaky_relu is positively homogeneous).
  - output is stored transposed ([128, N]: y_r^T on rows 0:64, y_i^T on
    64:128); the host untransposes, interleaves, adds bias, sums parities.

v2 structure (vs v1 baseline at 131us):
  - scores for r and i go into ONE 3D PSUM tile [128, 2, 512] (2 banks);
    the relu drain covers both components in ONE DVE/ACT op (fixed-cost
    amortized), alternating VectorE / ScalarE by a 1:2 pattern.
  - y_r^T and y_i^T share ONE [128, 512] PSUM bank: value matmuls are
    col-group packed (r -> partitions 0:64 via stationary cols 0:64,
    i -> partitions 64:128).  The PE runs such M=64 pairs concurrently in
    disjoint column groups, roughly halving value-matmul time.
  - diagonal blocks: the second diag j-block of each slot is fully masked
    for i-offsets < 256 on both core parities, so score matmuls, drains and
    value matmuls subrange to [256:512] there.
  - PSUM: 3 score bufs (6 banks) + 2 y bufs (2 banks) = 8 banks.

leaky_relu lowering (RELU_CORR): leaky(s) = 0.99*relu(s) + 0.01*s.  For
causally-full j-blocks the 0.01*s term telescopes into a per-slot constant
matmul: mcorr = 0.01*sum_J kp_J @ V'_J, precomputed on the host and
accumulated into the y PSUM bank.  Diagonal tiles compute u = mask*s
(VectorE, drains) and w = relu(u), feeding matmuls against 0.01*V' and
0.99*V'.

NOTE: ACT Lrelu reading PSUM hangs TRN2 (empirically) — never emit it.
"""

import numpy as np

import concourse.bacc as bacc
import concourse.tile as tile
from concourse import mybir
from concourse.bass_utils import run_bass_kernel_spmd

B, N, F = 4, 4096, 64
P = 128             # = 2*F: score contraction width / partition count
JB = 128            # j-block width
IBW = 512           # i-block (slot) width
NSLOT = N // IBW    # 8 slots
NJPAR = N // JB // 2  # 16 parity j-blocks per core
NEG = 0.01
SCALE = 1.0 / 64.0  # 1/sqrt(N)
NCORES = 8
DIAG1_OFF = 256     # both parities: 2nd diag block dead for i-offset < 256

_DT = mybir.dt.float32
MM_BF16 = True      # bf16 matmul inputs: half the DMA bytes, 4x fp32 PE rate
SIM_SAFE_LRELU = True  # kept for test.py compat (unused)
_CACHE: dict = {}


def _build_nc():
    nc = bacc.Bacc("TRN2", target_bir_lowering=False, num_devices=NCORES)
    dt = _DT
    mdt = mybir.dt.bfloat16 if MM_BF16 else _DT  # matmul input dtype
    qrT = nc.dram_tensor("qrT", [P, N], mdt, kind="ExternalInput")
    qiT = nc.dram_tensor("qiT", [P, N], mdt, kind="ExternalInput")
    # fused copy of q (dim1 = component) for the late chunks: one DMA
    # descriptor covers both components
    q2f = nc.dram_tensor("q2f", [P, 2, N], mdt, kind="ExternalInput")
    kp = nc.dram_tensor("kp", [P, NJPAR * JB], mdt, kind="ExternalInput")
    # va = 0.99 * V' (relu term), vb = 0.01 * V' (raw term, diagonal only);
    # dim1 = component
    va2 = nc.dram_tensor("va2", [P, 2, NJPAR * F], mdt, kind="ExternalInput")
    vb2 = nc.dram_tensor("vb2", [P, 2, NJPAR * F], mdt, kind="ExternalInput")
    # per-slot correction weights: 0.01 * sum_{full J} kp_J @ V'_J  [P, 64]
    mc2 = nc.dram_tensor("mc2", [P, 2, NSLOT * F], mdt, kind="ExternalInput")
    # one mask per diag block k (broadcast over components on device)
    dmask = nc.dram_tensor("dmask", [JB, 2, IBW], mdt, kind="ExternalInput")
    out = nc.dram_tensor("out", [P, N], mdt, kind="ExternalOutput")

    relu = mybir.ActivationFunctionType.Relu
    mul_op = mybir.AluOpType.mult
    max_op = mybir.AluOpType.max
    add_op = mybir.AluOpType.add

    with tile.TileContext(nc) as tc:
        with (
            tc.tile_pool(name="res", bufs=1) as res,
            tc.tile_pool(name="wp", bufs=3) as wp,
            tc.tile_pool(name="osb", bufs=2) as osb,
            tc.tile_pool(name="spsum", bufs=3, space="PSUM") as spsum,
            tc.tile_pool(name="ypsum", bufs=2, space="PSUM") as ypsum,
        ):
            # ---- input staging; order so slot 0's operands land first ----
            sb_q = res.tile([P, 2, N], mdt, tag="q")
            sb_k = res.tile([P, NJPAR * JB], mdt, tag="k")
            sb_m = res.tile([JB, 2, IBW], mdt, tag="m")
            sb_va = res.tile([P, 2, NJPAR * F], mdt, tag="va")
            sb_vb = res.tile([P, 2, NJPAR * F], mdt, tag="vb")
            sb_mc = res.tile([P, 2, NSLOT * F], mdt, tag="mc")

            # All input DMAs on the Sync queue (out-DMAs go to GpSimd),
            # ordered by first use under the diag-first slot schedule.
            # Early q chunks are split per component for latency; late ones
            # and all v/mcorr tensors are component-fused (1 descriptor).
            nc.sync.dma_start(out=sb_q[:, 0:1, 0:512], in_=qrT[:, 0:512])
            nc.sync.dma_start(out=sb_k[:, 0:256], in_=kp[:, 0:256])
            nc.sync.dma_start(out=sb_q[:, 1:2, 0:512], in_=qiT[:, 0:512])
            nc.sync.dma_start(out=sb_m[:, 0:1, :], in_=dmask[:, 0:1, :])
            nc.sync.dma_start(out=sb_vb[:, :, 0:256], in_=vb2[:, :, 0:256])
            nc.sync.dma_start(out=sb_m[:, 1:2, :], in_=dmask[:, 1:2, :])
            nc.sync.dma_start(out=sb_k[:, 256:512], in_=kp[:, 256:512])
            nc.sync.dma_start(out=sb_q[:, 0:1, 512:1024], in_=qrT[:, 512:1024])
            nc.sync.dma_start(out=sb_q[:, 1:2, 512:1024], in_=qiT[:, 512:1024])
            nc.sync.dma_start(out=sb_va[:, :, 0:256], in_=va2[:, :, 0:256])
            nc.sync.dma_start(out=sb_mc[:, :, 0:192], in_=mc2[:, :, 0:192])
            nc.sync.dma_start(out=sb_k[:, 512:1024], in_=kp[:, 512:1024])
            nc.sync.dma_start(out=sb_q[:, 0:1, 1024:1536], in_=qrT[:, 1024:1536])
            nc.sync.dma_start(out=sb_q[:, 1:2, 1024:1536], in_=qiT[:, 1024:1536])
            nc.sync.dma_start(out=sb_vb[:, :, 256:512], in_=vb2[:, :, 256:512])
            nc.sync.dma_start(out=sb_q[:, 0:1, 1536:2048], in_=qrT[:, 1536:2048])
            nc.sync.dma_start(out=sb_q[:, 1:2, 1536:2048], in_=qiT[:, 1536:2048])
            nc.sync.dma_start(out=sb_va[:, :, 256:512], in_=va2[:, :, 256:512])
            nc.sync.dma_start(out=sb_mc[:, :, 192:512], in_=mc2[:, :, 192:512])
            nc.sync.dma_start(out=sb_k[:, 1024:1536], in_=kp[:, 1024:1536])
            nc.sync.dma_start(out=sb_q[:, :, 2048:2560], in_=q2f[:, :, 2048:2560])
            nc.sync.dma_start(out=sb_vb[:, :, 512:1024], in_=vb2[:, :, 512:1024])
            nc.sync.dma_start(out=sb_q[:, :, 2560:3072], in_=q2f[:, :, 2560:3072])
            nc.sync.dma_start(out=sb_va[:, :, 512:1024], in_=va2[:, :, 512:1024])
            nc.sync.dma_start(out=sb_k[:, 1536:2048], in_=kp[:, 1536:2048])
            nc.sync.dma_start(out=sb_q[:, :, 3072:3584], in_=q2f[:, :, 3072:3584])
            nc.sync.dma_start(out=sb_q[:, :, 3584:4096], in_=q2f[:, :, 3584:4096])
            sb_masks = tuple(
                sb_m[:, k:k + 1, :].broadcast_to([JB, 2, IBW])
                for k in range(2))

            # Warm the PE (HAM clock gate) with dummy matmuls while the first
            # DMAs are in flight, sized to span the DMA head so real matmuls
            # start at 2.4 GHz without being queued behind the warm-up.
            warm_sb = res.tile([P, F], mdt, tag="warm")
            nc.vector.memset(warm_sb[:], 0.0)
            warm_y = ypsum.tile([P, IBW], dt, tag="y")
            for _ in range(18):
                nc.tensor.matmul(warm_y[0:64, 0:64], warm_sb[:], warm_sb[:],
                                 start=True, stop=True)

            drain_ctr = 0  # full-tile relu drains: cycle V,S,S,S
            for s in range(NSLOT):
                cnt = 2 * s + 2
                nfull = cnt - 2
                isl = slice(s * IBW, (s + 1) * IBW)
                y = ypsum.tile([P, IBW], dt, tag="y")

                def sc_pair(p, o=0):
                    """Score matmul pair for j-block p into a fresh PSUM tile."""
                    ksl = slice(p * JB, (p + 1) * JB)
                    qsl = slice(s * IBW + o, (s + 1) * IBW)
                    s2 = spsum.tile([P, 2, IBW], dt, tag="s", name="s2")
                    nc.tensor.matmul(s2[:, 0:1, o:], sb_k[:, ksl],
                                     sb_q[:, 0:1, qsl], start=True, stop=True)
                    nc.tensor.matmul(s2[:, 1:2, o:], sb_k[:, ksl],
                                     sb_q[:, 1:2, qsl], start=True, stop=True)
                    return s2

                def val_pair(p, w, start):
                    vsl = slice(p * F, (p + 1) * F)
                    nc.tensor.matmul(y[0:64, :], sb_va[:, 0:1, vsl],
                                     w[:, 0:1, :], start=start, stop=False)
                    nc.tensor.matmul(y[64:128, :], sb_va[:, 1:2, vsl],
                                     w[:, 1:2, :], start=start, stop=False)

                # Full blocks, software-pipelined two deep: the value pair
                # for block p is emitted after block p+2's scores so its
                # drain has cover (the tensor queue is strict FIFO).
                pend = []  # (p, w) awaiting value pairs
                for p in range(nfull):
                    s2 = sc_pair(p)
                    w = wp.tile([P, 2, IBW], mdt, tag="w")
                    if drain_ctr % 4 == 0:
                        nc.vector.tensor_scalar_max(w[:], s2[:], 0.0)
                    else:
                        nc.scalar.activation(w[:], s2[:], relu)
                    drain_ctr += 1
                    if len(pend) == 2:
                        q0 = pend.pop(0)
                        val_pair(q0[0], q0[1], q0[0] == 0)
                    pend.append((p, w))

                # Diagonal blocks: u = mask*s (drains), w2 = max(100u, u) =
                # u + 99*relu(u), so vb @ w2 = 0.01*V'*u + 0.99*V'*relu(u).
                w2s = []
                for k in range(2):
                    p = nfull + k
                    o = DIAG1_OFF if k == 1 else 0
                    s2 = sc_pair(p, o)
                    mk = sb_masks[k]
                    u = wp.tile([P, 2, IBW], mdt, tag="u", bufs=2)
                    nc.vector.tensor_tensor(out=u[:, :, o:], in0=s2[:, :, o:],
                                            in1=mk[:, :, o:], op=mul_op)
                    w2 = wp.tile([P, 2, IBW], mdt, tag="w2", bufs=4)
                    nc.vector.scalar_tensor_tensor(
                        out=w2[:, :, o:], in0=u[:, :, o:], scalar=100.0,
                        in1=u[:, :, o:], op0=mul_op, op1=max_op)
                    w2s.append((w2, o, p))
                    if pend:
                        q0 = pend.pop(0)
                        val_pair(q0[0], q0[1], q0[0] == 0)

                for q0 in pend:
                    val_pair(q0[0], q0[1], q0[0] == 0)
                pend = []

                # Correction pair (no drain dependency) covers the diag
                # chains' latency: y += (0.01 * sum_full kp_J @ V'_J)^T @ q
                if s > 0:
                    msl = slice(s * F, (s + 1) * F)
                    nc.tensor.matmul(y[0:64, :], sb_mc[:, 0:1, msl],
                                     sb_q[:, 0:1, isl], start=False, stop=False)
                    nc.tensor.matmul(y[64:128, :], sb_mc[:, 1:2, msl],
                                     sb_q[:, 1:2, isl], start=False, stop=False)

                # Diagonal vb pairs close the slot.
                for k, (w2, o, p) in enumerate(w2s):
                    vsl = slice(p * F, (p + 1) * F)
                    first = s == 0 and k == 0
                    last = k == 1
                    nc.tensor.matmul(y[0:64, o:], sb_vb[:, 0:1, vsl],
                                     w2[:, 0:1, o:], start=first, stop=last)
                    nc.tensor.matmul(y[64:128, o:], sb_vb[:, 1:2, vsl],
                                     w2[:, 1:2, o:], start=first, stop=last)

                # tail: accumulator to SBUF bf16 (alternate V/S), DMA out;
                # the final slot splits copy and DMA so its exposed tail is
                # as short as possible
                y_sb = osb.tile([P, IBW], mdt, tag="ysb")
                if s == NSLOT - 1:
                    nc.vector.tensor_copy(y_sb[0:64, :], y[0:64, :])
                    nc.scalar.copy(y_sb[64:128, :], y[64:128, :])
                    nc.gpsimd.dma_start(out=out[0:32, isl], in_=y_sb[0:32, :])
                    nc.sync.dma_start(out=out[32:64, isl], in_=y_sb[32:64, :])
                    nc.gpsimd.dma_start(out=out[64:96, isl], in_=y_sb[64:96, :])
                    nc.sync.dma_start(out=out[96:128, isl], in_=y_sb[96:128, :])
                elif s == NSLOT - 2:
                    nc.scalar.copy(y_sb[:], y[:])
                    nc.gpsimd.dma_start(out=out[0:64, isl], in_=y_sb[0:64, :])
                    nc.sync.dma_start(out=out[64:128, isl], in_=y_sb[64:128, :])
                else:
                    if s % 2 == 0:
                        nc.vector.tensor_copy(y_sb[:], y[:])
                    else:
                        nc.scalar.copy(y_sb[:], y[:])
                    nc.gpsimd.dma_start(out=out[:, isl], in_=y_sb[:])
    nc.compile()
    return nc


def _prep_inputs(Q, K, V, W_att, b_att):
    """Host-side re-layout: per-core in_maps for run_bass_kernel_spmd."""
    Q = np.asarray(Q, dtype=np.float32)
    K = np.asarray(K, dtype=np.float32)
    V = np.asarray(V, dtype=np.float32)
    W_att = np.asarray(W_att, dtype=np.float32)

    Qf = Q.reshape(B, N, P)          # [b, i, f*2+c]
    Kf = K.reshape(B, N, P)
    Vpr = SCALE * (V[..., 0] @ W_att.T)   # [B, N, F]
    Vpi = SCALE * (V[..., 1] @ W_att.T)

    # causal masks for a slot's last two parity j-blocks, per core parity h:
    # diagonal sub-block d = 2k+h of the slot's group of 4; duplicated along
    # a component axis -> [2, JB, 2, IBW]
    jj = np.arange(JB)[:, None]
    ii = np.arange(IBW)[None, :]
    masks = {}
    for h in (0, 1):
        masks[h] = np.stack([(ii >= jj + JB * (2 * k + h)).astype(np.float32)
                             for k in range(2)], axis=1)   # [JB, 2, IBW]

    if MM_BF16:
        import ml_dtypes
        cvt = lambda a: np.ascontiguousarray(a).astype(ml_dtypes.bfloat16)
    else:
        cvt = lambda a: np.ascontiguousarray(a, dtype=np.float32)

    in_maps = []
    for c in range(NCORES):
        b, h = divmod(c, 2)
        Qmodr = Qf[b].copy()
        Qmodr[:, 1::2] *= -1.0
        Qmodi = np.empty_like(Qf[b])
        Qmodi[:, 0::2] = Qf[b][:, 1::2]
        Qmodi[:, 1::2] = Qf[b][:, 0::2]
        # parity-packed K: [P, NJPAR*JB], position pp holds block J = 2*pp+h
        kp3 = Kf[b].reshape(N // JB, JB, P)[h::2]          # [16, j, p]
        kp = kp3.transpose(2, 0, 1).reshape(P, -1)         # [p, pp*JB+j]
        vr3 = Vpr[b].reshape(N // JB, JB, F)[h::2]         # [16, j, f]
        vi3 = Vpi[b].reshape(N // JB, JB, F)[h::2]
        vpr = vr3.transpose(1, 0, 2).reshape(JB, -1)       # [j, pp*F+f]
        vpi = vi3.transpose(1, 0, 2).reshape(JB, -1)
        # per-slot correction: 0.01 * sum over FULL blocks (pos < cnt-2 = 2s)
        prod_r = np.einsum('bjp,bjf->bpf', kp3, vr3)       # [16, p, f]
        prod_i = np.einsum('bjp,bjf->bpf', kp3, vi3)
        pre_r = np.concatenate(
            [np.zeros((1, P, F), np.float32), np.cumsum(prod_r, axis=0)])
        pre_i = np.concatenate(
            [np.zeros((1, P, F), np.float32), np.cumsum(prod_i, axis=0)])
        mcr = np.concatenate([NEG * pre_r[2 * s] for s in range(NSLOT)], axis=1)
        mci = np.concatenate([NEG * pre_i[2 * s] for s in range(NSLOT)], axis=1)
        qrt, qit = Qmodr.T, Qmodi.T
        in_maps.append({
            "qrT": cvt(qrt),
            "qiT": cvt(qit),
            "q2f": cvt(np.stack([qrt, qit], axis=1)),
            "kp": cvt(kp),
            "va2": cvt((1.0 - NEG) * np.stack([vpr, vpi], axis=1)),
            "vb2": cvt(NEG * np.stack([vpr, vpi], axis=1)),
            "mc2": cvt(np.stack([mcr, mci], axis=1)),
            "dmask": cvt(masks[h]),
        })
    return in_maps


def _gather(results, b_att):
    b_att = np.asarray(b_att, dtype=np.float32)
    out = np.empty((B, N, F, 2), dtype=np.float32)
    for b in range(B):
        y = (np.asarray(results[2 * b]["out"], dtype=np.float32)
             + np.asarray(results[2 * b + 1]["out"], dtype=np.float32))
        out[b, :, :, 0] = y[0:64].T + b_att[None, :]
        out[b, :, :, 1] = y[64:128].T + b_att[None, :]
    return out


def kernel(Q, K, V, W_att, b_att):
    if "nc" not in _CACHE:
        _CACHE["nc"] = _build_nc()
    nc = _CACHE["nc"]
    in_maps = _prep_inputs(Q, K, V, W_att, b_att)
    res = run_bass_kernel_spmd(nc, in_maps, core_ids=list(range(NCORES)))
    return _gather(res.results, b_att)


# revision 22
# speedup vs baseline: 1.1224x; 1.0067x over previous
"""Trainium2 Bass kernel for nn_AttentionOutput (complex causal leaky-relu attention).

Reference (B=4, N=4096, F=64), per batch:
    sr = (Qr@Kr^T - Qi@Ki^T)/sqrt(N); si = (Qr@Ki^T + Qi@Kr^T)/sqrt(N)
    wr = tril * leaky_relu(sr);        wi = tril * leaky_relu(si)
    out_r = (wr@Vr)@W_att^T + b;       out_i = (wi@Vi)@W_att^T + b

Distribution: 2 cores per batch.  Core parity h processes j-blocks J === h
(mod 2) for ALL 4096 query rows; causal work is then identical across cores
(slot I needs 2I+2 j-blocks), so a single SPMD program serves all 8 cores and
the host sums the two partial outputs per batch.

Host-side layout prep removes every on-device transpose:
  - scores contract over p = f*2+c (128 partitions, ONE matmul per component):
    sr = Qmodr . K^T where Qmodr = Q with odd columns negated, and
    si = Qmodi . K^T where Qmodi = Q with column pairs swapped; K stays plain.
    Both Q variants are fed pre-transposed [128, N].
  - V' = (1/64) V @ W_att^T folds the score scale and the output projection
    into the attention-value matmul (leaky_relu is positively homogeneous).
  - output is stored transposed ([128, N]: y_r^T on rows 0:64, y_i^T on
    64:128); the host untransposes, interleaves, adds bias, sums parities.

v2 structure (vs v1 baseline at 131us):
  - scores for r and i go into ONE 3D PSUM tile [128, 2, 512] (2 banks);
    the relu drain covers both components in ONE DVE/ACT op (fixed-cost
    amortized), alternating VectorE / ScalarE by a 1:2 pattern.
  - y_r^T and y_i^T share ONE [128, 512] PSUM bank: value matmuls are
    col-group packed (r -> partitions 0:64 via stationary cols 0:64,
    i -> partitions 64:128).  The PE runs such M=64 pairs concurrently in
    disjoint column groups, roughly halving value-matmul time.
  - diagonal blocks: the second diag j-block of each slot is fully masked
    for i-offsets < 256 on both core parities, so score matmuls, drains and
    value matmuls subrange to [256:512] there.
  - PSUM: 3 score bufs (6 banks) + 2 y bufs (2 banks) = 8 banks.

leaky_relu lowering (RELU_CORR): leaky(s) = 0.99*relu(s) + 0.01*s.  For
causally-full j-blocks the 0.01*s term telescopes into a per-slot constant
matmul: mcorr = 0.01*sum_J kp_J @ V'_J, precomputed on the host and
accumulated into the y PSUM bank.  Diagonal tiles compute u = mask*s
(VectorE, drains) and w = relu(u), feeding matmuls against 0.01*V' and
0.99*V'.

NOTE: ACT Lrelu reading PSUM hangs TRN2 (empirically) — never emit it.
"""

import numpy as np

import concourse.bacc as bacc
import concourse.tile as tile
from concourse import mybir
from concourse.bass_utils import run_bass_kernel_spmd

B, N, F = 4, 4096, 64
P = 128             # = 2*F: score contraction width / partition count
JB = 128            # j-block width
IBW = 512           # i-block (slot) width
NSLOT = N // IBW    # 8 slots
NJPAR = N // JB // 2  # 16 parity j-blocks per core
NEG = 0.01
SCALE = 1.0 / 64.0  # 1/sqrt(N)
NCORES = 8
DIAG1_OFF = 256     # both parities: 2nd diag block dead for i-offset < 256

_DT = mybir.dt.float32
MM_BF16 = True      # bf16 matmul inputs: half the DMA bytes, 4x fp32 PE rate
SIM_SAFE_LRELU = True  # kept for test.py compat (unused)
_CACHE: dict = {}


def _build_nc():
    nc = bacc.Bacc("TRN2", target_bir_lowering=False, num_devices=NCORES)
    dt = _DT
    mdt = mybir.dt.bfloat16 if MM_BF16 else _DT  # matmul input dtype
    qrT = nc.dram_tensor("qrT", [P, N], mdt, kind="ExternalInput")
    qiT = nc.dram_tensor("qiT", [P, N], mdt, kind="ExternalInput")
    # fused copy of q (dim1 = component) for the late chunks: one DMA
    # descriptor covers both components
    q2f = nc.dram_tensor("q2f", [P, 2, N], mdt, kind="ExternalInput")
    kp = nc.dram_tensor("kp", [P, NJPAR * JB], mdt, kind="ExternalInput")
    # va = 0.99 * V' (relu term), vb = 0.01 * V' (raw term, diagonal only);
    # dim1 = component
    va2 = nc.dram_tensor("va2", [P, 2, NJPAR * F], mdt, kind="ExternalInput")
    vb2 = nc.dram_tensor("vb2", [P, 2, NJPAR * F], mdt, kind="ExternalInput")
    # per-slot correction weights: 0.01 * sum_{full J} kp_J @ V'_J  [P, 64]
    mc2 = nc.dram_tensor("mc2", [P, 2, NSLOT * F], mdt, kind="ExternalInput")
    # one mask per diag block k (broadcast over components on device)
    dmask = nc.dram_tensor("dmask", [JB, 2, IBW], mdt, kind="ExternalInput")
    out = nc.dram_tensor("out", [P, N], mdt, kind="ExternalOutput")

    relu = mybir.ActivationFunctionType.Relu
    mul_op = mybir.AluOpType.mult
    max_op = mybir.AluOpType.max
    add_op = mybir.AluOpType.add

    with tile.TileContext(nc) as tc:
        with (
            tc.tile_pool(name="res", bufs=1) as res,
            tc.tile_pool(name="wp", bufs=3) as wp,
            tc.tile_pool(name="osb", bufs=2) as osb,
            tc.tile_pool(name="spsum", bufs=3, space="PSUM") as spsum,
            tc.tile_pool(name="ypsum", bufs=2, space="PSUM") as ypsum,
        ):
            # ---- input staging; order so slot 0's operands land first ----
            sb_q = res.tile([P, 2, N], mdt, tag="q")
            sb_k = res.tile([P, NJPAR * JB], mdt, tag="k")
            sb_m = res.tile([JB, 2, IBW], mdt, tag="m")
            sb_va = res.tile([P, 2, NJPAR * F], mdt, tag="va")
            sb_vb = res.tile([P, 2, NJPAR * F], mdt, tag="vb")
            sb_mc = res.tile([P, 2, NSLOT * F], mdt, tag="mc")

            # All input DMAs on the Sync queue (out-DMAs go to GpSimd),
            # ordered by first use under the diag-first slot schedule.
            # Early q chunks are split per component for latency; late ones
            # and all v/mcorr tensors are component-fused (1 descriptor).
            nc.sync.dma_start(out=sb_q[:, 0:1, 0:512], in_=qrT[:, 0:512])
            nc.sync.dma_start(out=sb_k[:, 0:256], in_=kp[:, 0:256])
            nc.sync.dma_start(out=sb_q[:, 1:2, 0:512], in_=qiT[:, 0:512])
            nc.sync.dma_start(out=sb_m[:, 0:1, :], in_=dmask[:, 0:1, :])
            nc.sync.dma_start(out=sb_q[:, 0:1, 512:1024], in_=qrT[:, 512:1024])
            nc.sync.dma_start(out=sb_m[:, 1:2, :], in_=dmask[:, 1:2, :])
            nc.sync.dma_start(out=sb_q[:, 1:2, 512:1024], in_=qiT[:, 512:1024])
            nc.sync.dma_start(out=sb_vb[:, :, 0:256], in_=vb2[:, :, 0:256])
            nc.sync.dma_start(out=sb_k[:, 256:512], in_=kp[:, 256:512])
            nc.sync.dma_start(out=sb_va[:, :, 0:256], in_=va2[:, :, 0:256])
            nc.sync.dma_start(out=sb_mc[:, :, 0:192], in_=mc2[:, :, 0:192])
            nc.sync.dma_start(out=sb_k[:, 512:1024], in_=kp[:, 512:1024])
            nc.sync.dma_start(out=sb_q[:, 0:1, 1024:1536], in_=qrT[:, 1024:1536])
            nc.sync.dma_start(out=sb_q[:, 1:2, 1024:1536], in_=qiT[:, 1024:1536])
            nc.sync.dma_start(out=sb_vb[:, :, 256:512], in_=vb2[:, :, 256:512])
            nc.sync.dma_start(out=sb_q[:, 0:1, 1536:2048], in_=qrT[:, 1536:2048])
            nc.sync.dma_start(out=sb_q[:, 1:2, 1536:2048], in_=qiT[:, 1536:2048])
            nc.sync.dma_start(out=sb_va[:, :, 256:512], in_=va2[:, :, 256:512])
            nc.sync.dma_start(out=sb_mc[:, :, 192:512], in_=mc2[:, :, 192:512])
            nc.sync.dma_start(out=sb_k[:, 1024:1536], in_=kp[:, 1024:1536])
            nc.sync.dma_start(out=sb_q[:, :, 2048:2560], in_=q2f[:, :, 2048:2560])
            nc.sync.dma_start(out=sb_vb[:, :, 512:1024], in_=vb2[:, :, 512:1024])
            nc.sync.dma_start(out=sb_q[:, :, 2560:3072], in_=q2f[:, :, 2560:3072])
            nc.sync.dma_start(out=sb_va[:, :, 512:1024], in_=va2[:, :, 512:1024])
            nc.sync.dma_start(out=sb_k[:, 1536:2048], in_=kp[:, 1536:2048])
            nc.sync.dma_start(out=sb_q[:, :, 3072:3584], in_=q2f[:, :, 3072:3584])
            nc.sync.dma_start(out=sb_q[:, :, 3584:4096], in_=q2f[:, :, 3584:4096])
            sb_masks = tuple(
                sb_m[:, k:k + 1, :].broadcast_to([JB, 2, IBW])
                for k in range(2))

            # Warm the PE (HAM clock gate) with dummy matmuls while the first
            # DMAs are in flight, sized to span the DMA head so real matmuls
            # start at 2.4 GHz without being queued behind the warm-up.
            warm_sb = res.tile([P, F], mdt, tag="warm")
            nc.vector.memset(warm_sb[:], 0.0)
            warm_y = ypsum.tile([P, IBW], dt, tag="y")
            for _ in range(18):
                nc.tensor.matmul(warm_y[0:64, 0:64], warm_sb[:], warm_sb[:],
                                 start=True, stop=True)

            drain_ctr = 0  # full-tile relu drains: cycle V,S,S,S
            for s in range(NSLOT):
                cnt = 2 * s + 2
                nfull = cnt - 2
                isl = slice(s * IBW, (s + 1) * IBW)
                y = ypsum.tile([P, IBW], dt, tag="y")

                def sc_pair(p, o=0):
                    """Score matmul pair for j-block p into a fresh PSUM tile."""
                    ksl = slice(p * JB, (p + 1) * JB)
                    qsl = slice(s * IBW + o, (s + 1) * IBW)
                    s2 = spsum.tile([P, 2, IBW], dt, tag="s", name="s2")
                    nc.tensor.matmul(s2[:, 0:1, o:], sb_k[:, ksl],
                                     sb_q[:, 0:1, qsl], start=True, stop=True)
                    nc.tensor.matmul(s2[:, 1:2, o:], sb_k[:, ksl],
                                     sb_q[:, 1:2, qsl], start=True, stop=True)
                    return s2

                def val_pair(p, w, start):
                    vsl = slice(p * F, (p + 1) * F)
                    nc.tensor.matmul(y[0:64, :], sb_va[:, 0:1, vsl],
                                     w[:, 0:1, :], start=start, stop=False)
                    nc.tensor.matmul(y[64:128, :], sb_va[:, 1:2, vsl],
                                     w[:, 1:2, :], start=start, stop=False)

                # Full blocks, software-pipelined two deep: the value pair
                # for block p is emitted after block p+2's scores so its
                # drain has cover (the tensor queue is strict FIFO).
                pend = []  # (p, w) awaiting value pairs
                for p in range(nfull):
                    s2 = sc_pair(p)
                    w = wp.tile([P, 2, IBW], mdt, tag="w")
                    # Small slots drain on ScalarE only (VectorE is busy with
                    # their diag chains); big slots give 1/3 to VectorE.
                    if s >= 5 and p % 3 == 1:
                        nc.vector.tensor_scalar_max(w[:], s2[:], 0.0)
                    else:
                        nc.scalar.activation(w[:], s2[:], relu)
                    if len(pend) == 3:
                        q0 = pend.pop(0)
                        val_pair(q0[0], q0[1], q0[0] == 0)
                    pend.append((p, w))

                # Diagonal blocks: u = mask*s (drains), w2 = max(100u, u) =
                # u + 99*relu(u), so vb @ w2 = 0.01*V'*u + 0.99*V'*relu(u).
                w2s = []
                for k in range(2):
                    p = nfull + k
                    o = DIAG1_OFF if k == 1 else 0
                    s2 = sc_pair(p, o)
                    mk = sb_masks[k]
                    u = wp.tile([P, 2, IBW], mdt, tag="u", bufs=2)
                    nc.vector.tensor_tensor(out=u[:, :, o:], in0=s2[:, :, o:],
                                            in1=mk[:, :, o:], op=mul_op)
                    w2 = wp.tile([P, 2, IBW], mdt, tag="w2", bufs=4)
                    nc.vector.scalar_tensor_tensor(
                        out=w2[:, :, o:], in0=u[:, :, o:], scalar=100.0,
                        in1=u[:, :, o:], op0=mul_op, op1=max_op)
                    w2s.append((w2, o, p))
                    if pend:
                        q0 = pend.pop(0)
                        val_pair(q0[0], q0[1], q0[0] == 0)

                for q0 in pend:
                    val_pair(q0[0], q0[1], q0[0] == 0)
                pend = []

                # Correction pair (no drain dependency) covers the diag
                # chains' latency: y += (0.01 * sum_full kp_J @ V'_J)^T @ q
                if s > 0:
                    msl = slice(s * F, (s + 1) * F)
                    nc.tensor.matmul(y[0:64, :], sb_mc[:, 0:1, msl],
                                     sb_q[:, 0:1, isl], start=False, stop=False)
                    nc.tensor.matmul(y[64:128, :], sb_mc[:, 1:2, msl],
                                     sb_q[:, 1:2, isl], start=False, stop=False)

                # Diagonal vb pairs close the slot.
                for k, (w2, o, p) in enumerate(w2s):
                    vsl = slice(p * F, (p + 1) * F)
                    first = s == 0 and k == 0
                    last = k == 1
                    nc.tensor.matmul(y[0:64, o:], sb_vb[:, 0:1, vsl],
                                     w2[:, 0:1, o:], start=first, stop=last)
                    nc.tensor.matmul(y[64:128, o:], sb_vb[:, 1:2, vsl],
                                     w2[:, 1:2, o:], start=first, stop=last)

                # tail: accumulator to SBUF bf16 (alternate V/S), DMA out;
                # the final slot splits copy and DMA so its exposed tail is
                # as short as possible
                y_sb = osb.tile([P, IBW], mdt, tag="ysb")
                if s == NSLOT - 1:
                    nc.vector.tensor_copy(y_sb[0:64, :], y[0:64, :])
                    nc.scalar.copy(y_sb[64:128, :], y[64:128, :])
                    nc.gpsimd.dma_start(out=out[0:32, isl], in_=y_sb[0:32, :])
                    nc.sync.dma_start(out=out[32:64, isl], in_=y_sb[32:64, :])
                    nc.gpsimd.dma_start(out=out[64:96, isl], in_=y_sb[64:96, :])
                    nc.sync.dma_start(out=out[96:128, isl], in_=y_sb[96:128, :])
                elif s == NSLOT - 2:
                    nc.scalar.copy(y_sb[:], y[:])
                    nc.gpsimd.dma_start(out=out[0:64, isl], in_=y_sb[0:64, :])
                    nc.sync.dma_start(out=out[64:128, isl], in_=y_sb[64:128, :])
                else:
                    if s % 2 == 0:
                        nc.vector.tensor_copy(y_sb[:], y[:])
                    else:
                        nc.scalar.copy(y_sb[:], y[:])
                    nc.gpsimd.dma_start(out=out[:, isl], in_=y_sb[:])
    nc.compile()
    return nc


def _prep_inputs(Q, K, V, W_att, b_att):
    """Host-side re-layout: per-core in_maps for run_bass_kernel_spmd."""
    Q = np.asarray(Q, dtype=np.float32)
    K = np.asarray(K, dtype=np.float32)
    V = np.asarray(V, dtype=np.float32)
    W_att = np.asarray(W_att, dtype=np.float32)

    Qf = Q.reshape(B, N, P)          # [b, i, f*2+c]
    Kf = K.reshape(B, N, P)
    Vpr = SCALE * (V[..., 0] @ W_att.T)   # [B, N, F]
    Vpi = SCALE * (V[..., 1] @ W_att.T)

    # causal masks for a slot's last two parity j-blocks, per core parity h:
    # diagonal sub-block d = 2k+h of the slot's group of 4; duplicated along
    # a component axis -> [2, JB, 2, IBW]
    jj = np.arange(JB)[:, None]
    ii = np.arange(IBW)[None, :]
    masks = {}
    for h in (0, 1):
        masks[h] = np.stack([(ii >= jj + JB * (2 * k + h)).astype(np.float32)
                             for k in range(2)], axis=1)   # [JB, 2, IBW]

    if MM_BF16:
        import ml_dtypes
        cvt = lambda a: np.ascontiguousarray(a).astype(ml_dtypes.bfloat16)
    else:
        cvt = lambda a: np.ascontiguousarray(a, dtype=np.float32)

    in_maps = []
    for c in range(NCORES):
        b, h = divmod(c, 2)
        Qmodr = Qf[b].copy()
        Qmodr[:, 1::2] *= -1.0
        Qmodi = np.empty_like(Qf[b])
        Qmodi[:, 0::2] = Qf[b][:, 1::2]
        Qmodi[:, 1::2] = Qf[b][:, 0::2]
        # parity-packed K: [P, NJPAR*JB], position pp holds block J = 2*pp+h
        kp3 = Kf[b].reshape(N // JB, JB, P)[h::2]          # [16, j, p]
        kp = kp3.transpose(2, 0, 1).reshape(P, -1)         # [p, pp*JB+j]
        vr3 = Vpr[b].reshape(N // JB, JB, F)[h::2]         # [16, j, f]
        vi3 = Vpi[b].reshape(N // JB, JB, F)[h::2]
        vpr = vr3.transpose(1, 0, 2).reshape(JB, -1)       # [j, pp*F+f]
        vpi = vi3.transpose(1, 0, 2).reshape(JB, -1)
        # per-slot correction: 0.01 * sum over FULL blocks (pos < cnt-2 = 2s)
        prod_r = np.einsum('bjp,bjf->bpf', kp3, vr3)       # [16, p, f]
        prod_i = np.einsum('bjp,bjf->bpf', kp3, vi3)
        pre_r = np.concatenate(
            [np.zeros((1, P, F), np.float32), np.cumsum(prod_r, axis=0)])
        pre_i = np.concatenate(
            [np.zeros((1, P, F), np.float32), np.cumsum(prod_i, axis=0)])
        mcr = np.concatenate([NEG * pre_r[2 * s] for s in range(NSLOT)], axis=1)
        mci = np.concatenate([NEG * pre_i[2 * s] for s in range(NSLOT)], axis=1)
        qrt, qit = Qmodr.T, Qmodi.T
        in_maps.append({
            "qrT": cvt(qrt),
            "qiT": cvt(qit),
            "q2f": cvt(np.stack([qrt, qit], axis=1)),
            "kp": cvt(kp),
            "va2": cvt((1.0 - NEG) * np.stack([vpr, vpi], axis=1)),
            "vb2": cvt(NEG * np.stack([vpr, vpi], axis=1)),
            "mc2": cvt(np.stack([mcr, mci], axis=1)),
            "dmask": cvt(masks[h]),
        })
    return in_maps


def _gather(results, b_att):
    b_att = np.asarray(b_att, dtype=np.float32)
    out = np.empty((B, N, F, 2), dtype=np.float32)
    for b in range(B):
        y = (np.asarray(results[2 * b]["out"], dtype=np.float32)
             + np.asarray(results[2 * b + 1]["out"], dtype=np.float32))
        out[b, :, :, 0] = y[0:64].T + b_att[None, :]
        out[b, :, :, 1] = y[64:128].T + b_att[None, :]
    return out


def kernel(Q, K, V, W_att, b_att):
    if "nc" not in _CACHE:
        _CACHE["nc"] = _build_nc()
    nc = _CACHE["nc"]
    in_maps = _prep_inputs(Q, K, V, W_att, b_att)
    res = run_bass_kernel_spmd(nc, in_maps, core_ids=list(range(NCORES)))
    return _gather(res.results, b_att)


# revision 23
# speedup vs baseline: 1.1531x; 1.0273x over previous
"""Trainium2 Bass kernel for nn_AttentionOutput (complex causal leaky-relu attention).

Reference (B=4, N=4096, F=64), per batch:
    sr = (Qr@Kr^T - Qi@Ki^T)/sqrt(N); si = (Qr@Ki^T + Qi@Kr^T)/sqrt(N)
    wr = tril * leaky_relu(sr);        wi = tril * leaky_relu(si)
    out_r = (wr@Vr)@W_att^T + b;       out_i = (wi@Vi)@W_att^T + b

Distribution: 2 cores per batch.  Core parity h processes j-blocks J === h
(mod 2) for ALL 4096 query rows; causal work is then identical across cores
(slot I needs 2I+2 j-blocks), so a single SPMD program serves all 8 cores and
the host sums the two partial outputs per batch.

Host-side layout prep removes every on-device transpose:
  - scores contract over p = f*2+c (128 partitions, ONE matmul per component):
    sr = Qmodr . K^T where Qmodr = Q with odd columns negated, and
    si = Qmodi . K^T where Qmodi = Q with column pairs swapped; K stays plain.
    Both Q variants are fed pre-transposed [128, N].
  - V' = (1/64) V @ W_att^T folds the score scale and the output projection
    into the attention-value matmul (leaky_relu is positively homogeneous).
  - output is stored transposed ([128, N]: y_r^T on rows 0:64, y_i^T on
    64:128); the host untransposes, interleaves, adds bias, sums parities.

v2 structure (vs v1 baseline at 131us):
  - scores for r and i go into ONE 3D PSUM tile [128, 2, 512] (2 banks);
    the relu drain covers both components in ONE DVE/ACT op (fixed-cost
    amortized), alternating VectorE / ScalarE by a 1:2 pattern.
  - y_r^T and y_i^T share ONE [128, 512] PSUM bank: value matmuls are
    col-group packed (r -> partitions 0:64 via stationary cols 0:64,
    i -> partitions 64:128).  The PE runs such M=64 pairs concurrently in
    disjoint column groups, roughly halving value-matmul time.
  - diagonal blocks: the second diag j-block of each slot is fully masked
    for i-offsets < 256 on both core parities, so score matmuls, drains and
    value matmuls subrange to [256:512] there.
  - PSUM: 3 score bufs (6 banks) + 2 y bufs (2 banks) = 8 banks.

leaky_relu lowering (RELU_CORR): leaky(s) = 0.99*relu(s) + 0.01*s.  For
causally-full j-blocks the 0.01*s term telescopes into a per-slot constant
matmul: mcorr = 0.01*sum_J kp_J @ V'_J, precomputed on the host and
accumulated into the y PSUM bank.  Diagonal tiles compute u = mask*s
(VectorE, drains) and w = relu(u), feeding matmuls against 0.01*V' and
0.99*V'.

NOTE: ACT Lrelu reading PSUM hangs TRN2 (empirically) — never emit it.
"""

import numpy as np

import concourse.bacc as bacc
import concourse.tile as tile
from concourse import mybir
from concourse.bass_utils import run_bass_kernel_spmd

B, N, F = 4, 4096, 64
P = 128             # = 2*F: score contraction width / partition count
JB = 128            # j-block width
IBW = 512           # i-block (slot) width
NSLOT = N // IBW    # 8 slots
NJPAR = N // JB // 2  # 16 parity j-blocks per core
NEG = 0.01
SCALE = 1.0 / 64.0  # 1/sqrt(N)
NCORES = 8
DIAG1_OFF = 256     # both parities: 2nd diag block dead for i-offset < 256

_DT = mybir.dt.float32
MM_BF16 = True      # bf16 matmul inputs: half the DMA bytes, 4x fp32 PE rate
SIM_SAFE_LRELU = True  # kept for test.py compat (unused)
_CACHE: dict = {}


def _build_nc():
    nc = bacc.Bacc("TRN2", target_bir_lowering=False, num_devices=NCORES)
    dt = _DT
    mdt = mybir.dt.bfloat16 if MM_BF16 else _DT  # matmul input dtype
    qrT = nc.dram_tensor("qrT", [P, N], mdt, kind="ExternalInput")
    qiT = nc.dram_tensor("qiT", [P, N], mdt, kind="ExternalInput")
    # fused copy of q (dim1 = component) for the late chunks: one DMA
    # descriptor covers both components
    q2f = nc.dram_tensor("q2f", [P, 2, N], mdt, kind="ExternalInput")
    kp = nc.dram_tensor("kp", [P, NJPAR * JB], mdt, kind="ExternalInput")
    # va = 0.99 * V' (relu term), vb = 0.01 * V' (raw term, diagonal only);
    # dim1 = component
    va2 = nc.dram_tensor("va2", [P, 2, NJPAR * F], mdt, kind="ExternalInput")
    vb2 = nc.dram_tensor("vb2", [P, 2, NJPAR * F], mdt, kind="ExternalInput")
    # per-slot correction weights: 0.01 * sum_{full J} kp_J @ V'_J  [P, 64]
    mc2 = nc.dram_tensor("mc2", [P, 2, NSLOT * F], mdt, kind="ExternalInput")
    # one mask per diag block k (broadcast over components on device)
    dmask = nc.dram_tensor("dmask", [JB, 2, IBW], mdt, kind="ExternalInput")
    out = nc.dram_tensor("out", [P, N], mdt, kind="ExternalOutput")

    relu = mybir.ActivationFunctionType.Relu
    mul_op = mybir.AluOpType.mult
    max_op = mybir.AluOpType.max
    add_op = mybir.AluOpType.add

    with tile.TileContext(nc) as tc:
        with (
            tc.tile_pool(name="res", bufs=1) as res,
            tc.tile_pool(name="wp", bufs=3) as wp,
            tc.tile_pool(name="osb", bufs=2) as osb,
            tc.tile_pool(name="spsum", bufs=3, space="PSUM") as spsum,
            tc.tile_pool(name="ypsum", bufs=2, space="PSUM") as ypsum,
        ):
            # ---- input staging; order so slot 0's operands land first ----
            sb_q = res.tile([P, 2, N], mdt, tag="q")
            sb_k = res.tile([P, NJPAR * JB], mdt, tag="k")
            sb_m = res.tile([JB, 2, IBW], mdt, tag="m")
            sb_va = res.tile([P, 2, NJPAR * F], mdt, tag="va")
            sb_vb = res.tile([P, 2, NJPAR * F], mdt, tag="vb")
            sb_mc = res.tile([P, 2, NSLOT * F], mdt, tag="mc")

            # All input DMAs on the Sync queue (out-DMAs go to GpSimd),
            # ordered by first use under the diag-first slot schedule.
            # Early q chunks are split per component for latency; late ones
            # and all v/mcorr tensors are component-fused (1 descriptor).
            nc.sync.dma_start(out=sb_q[:, 0:1, 0:512], in_=qrT[:, 0:512])
            nc.sync.dma_start(out=sb_k[:, 0:256], in_=kp[:, 0:256])
            nc.sync.dma_start(out=sb_q[:, 1:2, 0:512], in_=qiT[:, 0:512])
            nc.sync.dma_start(out=sb_m[:, 0:1, :], in_=dmask[:, 0:1, :])
            nc.sync.dma_start(out=sb_q[:, 0:1, 512:1024], in_=qrT[:, 512:1024])
            nc.sync.dma_start(out=sb_m[:, 1:2, :], in_=dmask[:, 1:2, :])
            nc.sync.dma_start(out=sb_q[:, 1:2, 512:1024], in_=qiT[:, 512:1024])
            nc.sync.dma_start(out=sb_vb[:, :, 0:256], in_=vb2[:, :, 0:256])
            nc.sync.dma_start(out=sb_k[:, 256:512], in_=kp[:, 256:512])
            nc.sync.dma_start(out=sb_va[:, :, 0:256], in_=va2[:, :, 0:256])
            nc.sync.dma_start(out=sb_mc[:, :, 0:192], in_=mc2[:, :, 0:192])
            nc.sync.dma_start(out=sb_k[:, 512:1024], in_=kp[:, 512:1024])
            nc.sync.dma_start(out=sb_q[:, 0:1, 1024:1536], in_=qrT[:, 1024:1536])
            nc.sync.dma_start(out=sb_q[:, 1:2, 1024:1536], in_=qiT[:, 1024:1536])
            nc.sync.dma_start(out=sb_vb[:, :, 256:512], in_=vb2[:, :, 256:512])
            nc.sync.dma_start(out=sb_q[:, 0:1, 1536:2048], in_=qrT[:, 1536:2048])
            nc.sync.dma_start(out=sb_q[:, 1:2, 1536:2048], in_=qiT[:, 1536:2048])
            nc.sync.dma_start(out=sb_va[:, :, 256:512], in_=va2[:, :, 256:512])
            nc.sync.dma_start(out=sb_mc[:, :, 192:512], in_=mc2[:, :, 192:512])
            nc.sync.dma_start(out=sb_k[:, 1024:1536], in_=kp[:, 1024:1536])
            nc.sync.dma_start(out=sb_q[:, :, 2048:2560], in_=q2f[:, :, 2048:2560])
            nc.sync.dma_start(out=sb_vb[:, :, 512:1024], in_=vb2[:, :, 512:1024])
            nc.sync.dma_start(out=sb_q[:, :, 2560:3072], in_=q2f[:, :, 2560:3072])
            nc.sync.dma_start(out=sb_va[:, :, 512:1024], in_=va2[:, :, 512:1024])
            nc.sync.dma_start(out=sb_k[:, 1536:2048], in_=kp[:, 1536:2048])
            nc.sync.dma_start(out=sb_q[:, :, 3072:3584], in_=q2f[:, :, 3072:3584])
            nc.sync.dma_start(out=sb_q[:, :, 3584:4096], in_=q2f[:, :, 3584:4096])
            sb_masks = tuple(
                sb_m[:, k:k + 1, :].broadcast_to([JB, 2, IBW])
                for k in range(2))

            # Warm the PE (HAM clock gate) with dummy matmuls while the first
            # DMAs are in flight, sized to span the DMA head so real matmuls
            # start at 2.4 GHz without being queued behind the warm-up.
            warm_sb = res.tile([P, F], mdt, tag="warm")
            nc.vector.memset(warm_sb[:], 0.0)
            warm_y = ypsum.tile([P, IBW], dt, tag="y")
            for _ in range(18):
                nc.tensor.matmul(warm_y[0:64, 0:64], warm_sb[:], warm_sb[:],
                                 start=True, stop=True)

            drain_ctr = 0  # full-tile relu drains: cycle V,S,S,S
            for s in range(NSLOT):
                cnt = 2 * s + 2
                nfull = cnt - 2
                isl = slice(s * IBW, (s + 1) * IBW)
                y = ypsum.tile([P, IBW], dt, tag="y")

                def sc_pair(p, o=0):
                    """Score matmul pair for j-block p into a fresh PSUM tile."""
                    ksl = slice(p * JB, (p + 1) * JB)
                    qsl = slice(s * IBW + o, (s + 1) * IBW)
                    s2 = spsum.tile([P, 2, IBW], dt, tag="s", name="s2")
                    nc.tensor.matmul(s2[:, 0:1, o:], sb_k[:, ksl],
                                     sb_q[:, 0:1, qsl], start=True, stop=True)
                    nc.tensor.matmul(s2[:, 1:2, o:], sb_k[:, ksl],
                                     sb_q[:, 1:2, qsl], start=True, stop=True)
                    return s2

                def val_pair(p, w, start):
                    vsl = slice(p * F, (p + 1) * F)
                    nc.tensor.matmul(y[0:64, :], sb_va[:, 0:1, vsl],
                                     w[:, 0:1, :], start=start, stop=False)
                    nc.tensor.matmul(y[64:128, :], sb_va[:, 1:2, vsl],
                                     w[:, 1:2, :], start=start, stop=False)

                # Full blocks, software-pipelined two deep: the value pair
                # for block p is emitted after block p+2's scores so its
                # drain has cover (the tensor queue is strict FIFO).
                pend = []  # (p, w) awaiting value pairs
                for p in range(nfull):
                    s2 = sc_pair(p)
                    w = wp.tile([P, 2, IBW], mdt, tag="w")
                    # Small slots drain on ScalarE only (VectorE is busy with
                    # their diag chains); big slots give 1/3 to VectorE.
                    if s >= 5 and p % 3 == 1 and p <= nfull - 3:
                        nc.vector.tensor_scalar_max(w[:], s2[:], 0.0)
                    else:
                        nc.scalar.activation(w[:], s2[:], relu)
                    if len(pend) == 3:
                        q0 = pend.pop(0)
                        val_pair(q0[0], q0[1], q0[0] == 0)
                    pend.append((p, w))

                # Diagonal blocks: u = mask*s (drains), w2 = max(100u, u) =
                # u + 99*relu(u), so vb @ w2 = 0.01*V'*u + 0.99*V'*relu(u).
                w2s = []
                for k in range(2):
                    p = nfull + k
                    o = DIAG1_OFF if k == 1 else 0
                    s2 = sc_pair(p, o)
                    mk = sb_masks[k]
                    u = wp.tile([P, 2, IBW], mdt, tag="u", bufs=2)
                    nc.vector.tensor_tensor(out=u[:, :, o:], in0=s2[:, :, o:],
                                            in1=mk[:, :, o:], op=mul_op)
                    w2 = wp.tile([P, 2, IBW], mdt, tag="w2", bufs=4)
                    nc.vector.scalar_tensor_tensor(
                        out=w2[:, :, o:], in0=u[:, :, o:], scalar=100.0,
                        in1=u[:, :, o:], op0=mul_op, op1=max_op)
                    w2s.append((w2, o, p))
                    if pend:
                        q0 = pend.pop(0)
                        val_pair(q0[0], q0[1], q0[0] == 0)

                for q0 in pend:
                    val_pair(q0[0], q0[1], q0[0] == 0)
                pend = []

                # Correction pair (no drain dependency) covers the diag
                # chains' latency: y += (0.01 * sum_full kp_J @ V'_J)^T @ q
                if s > 0:
                    msl = slice(s * F, (s + 1) * F)
                    nc.tensor.matmul(y[0:64, :], sb_mc[:, 0:1, msl],
                                     sb_q[:, 0:1, isl], start=False, stop=False)
                    nc.tensor.matmul(y[64:128, :], sb_mc[:, 1:2, msl],
                                     sb_q[:, 1:2, isl], start=False, stop=False)

                # Diagonal vb pairs close the slot.
                for k, (w2, o, p) in enumerate(w2s):
                    vsl = slice(p * F, (p + 1) * F)
                    first = s == 0 and k == 0
                    last = k == 1
                    nc.tensor.matmul(y[0:64, o:], sb_vb[:, 0:1, vsl],
                                     w2[:, 0:1, o:], start=first, stop=last)
                    nc.tensor.matmul(y[64:128, o:], sb_vb[:, 1:2, vsl],
                                     w2[:, 1:2, o:], start=first, stop=last)

                # tail: accumulator to SBUF bf16 (alternate V/S), DMA out;
                # the final slot splits copy and DMA so its exposed tail is
                # as short as possible
                y_sb = osb.tile([P, IBW], mdt, tag="ysb")
                if s == NSLOT - 1:
                    nc.vector.tensor_copy(y_sb[0:64, :], y[0:64, :])
                    nc.scalar.copy(y_sb[64:128, :], y[64:128, :])
                    nc.gpsimd.dma_start(out=out[0:32, isl], in_=y_sb[0:32, :])
                    nc.sync.dma_start(out=out[32:64, isl], in_=y_sb[32:64, :])
                    nc.gpsimd.dma_start(out=out[64:96, isl], in_=y_sb[64:96, :])
                    nc.sync.dma_start(out=out[96:128, isl], in_=y_sb[96:128, :])
                elif s == NSLOT - 2:
                    nc.scalar.copy(y_sb[:], y[:])
                    nc.gpsimd.dma_start(out=out[0:64, isl], in_=y_sb[0:64, :])
                    nc.sync.dma_start(out=out[64:128, isl], in_=y_sb[64:128, :])
                else:
                    if s % 2 == 0:
                        nc.vector.tensor_copy(y_sb[:], y[:])
                    else:
                        nc.scalar.copy(y_sb[:], y[:])
                    nc.gpsimd.dma_start(out=out[:, isl], in_=y_sb[:])
    nc.compile()
    return nc


def _prep_inputs(Q, K, V, W_att, b_att):
    """Host-side re-layout: per-core in_maps for run_bass_kernel_spmd."""
    Q = np.asarray(Q, dtype=np.float32)
    K = np.asarray(K, dtype=np.float32)
    V = np.asarray(V, dtype=np.float32)
    W_att = np.asarray(W_att, dtype=np.float32)

    Qf = Q.reshape(B, N, P)          # [b, i, f*2+c]
    Kf = K.reshape(B, N, P)
    Vpr = SCALE * (V[..., 0] @ W_att.T)   # [B, N, F]
    Vpi = SCALE * (V[..., 1] @ W_att.T)

    # causal masks for a slot's last two parity j-blocks, per core parity h:
    # diagonal sub-block d = 2k+h of the slot's group of 4; duplicated along
    # a component axis -> [2, JB, 2, IBW]
    jj = np.arange(JB)[:, None]
    ii = np.arange(IBW)[None, :]
    masks = {}
    for h in (0, 1):
        masks[h] = np.stack([(ii >= jj + JB * (2 * k + h)).astype(np.float32)
                             for k in range(2)], axis=1)   # [JB, 2, IBW]

    if MM_BF16:
        import ml_dtypes
        cvt = lambda a: np.ascontiguousarray(a).astype(ml_dtypes.bfloat16)
    else:
        cvt = lambda a: np.ascontiguousarray(a, dtype=np.float32)

    in_maps = []
    for c in range(NCORES):
        b, h = divmod(c, 2)
        Qmodr = Qf[b].copy()
        Qmodr[:, 1::2] *= -1.0
        Qmodi = np.empty_like(Qf[b])
        Qmodi[:, 0::2] = Qf[b][:, 1::2]
        Qmodi[:, 1::2] = Qf[b][:, 0::2]
        # parity-packed K: [P, NJPAR*JB], position pp holds block J = 2*pp+h
        kp3 = Kf[b].reshape(N // JB, JB, P)[h::2]          # [16, j, p]
        kp = kp3.transpose(2, 0, 1).reshape(P, -1)         # [p, pp*JB+j]
        vr3 = Vpr[b].reshape(N // JB, JB, F)[h::2]         # [16, j, f]
        vi3 = Vpi[b].reshape(N // JB, JB, F)[h::2]
        vpr = vr3.transpose(1, 0, 2).reshape(JB, -1)       # [j, pp*F+f]
        vpi = vi3.transpose(1, 0, 2).reshape(JB, -1)
        # per-slot correction: 0.01 * sum over FULL blocks (pos < cnt-2 = 2s)
        prod_r = np.einsum('bjp,bjf->bpf', kp3, vr3)       # [16, p, f]
        prod_i = np.einsum('bjp,bjf->bpf', kp3, vi3)
        pre_r = np.concatenate(
            [np.zeros((1, P, F), np.float32), np.cumsum(prod_r, axis=0)])
        pre_i = np.concatenate(
            [np.zeros((1, P, F), np.float32), np.cumsum(prod_i, axis=0)])
        mcr = np.concatenate([NEG * pre_r[2 * s] for s in range(NSLOT)], axis=1)
        mci = np.concatenate([NEG * pre_i[2 * s] for s in range(NSLOT)], axis=1)
        qrt, qit = Qmodr.T, Qmodi.T
        in_maps.append({
            "qrT": cvt(qrt),
            "qiT": cvt(qit),
            "q2f": cvt(np.stack([qrt, qit], axis=1)),
            "kp": cvt(kp),
            "va2": cvt((1.0 - NEG) * np.stack([vpr, vpi], axis=1)),
            "vb2": cvt(NEG * np.stack([vpr, vpi], axis=1)),
            "mc2": cvt(np.stack([mcr, mci], axis=1)),
            "dmask": cvt(masks[h]),
        })
    return in_maps


def _gather(results, b_att):
    b_att = np.asarray(b_att, dtype=np.float32)
    out = np.empty((B, N, F, 2), dtype=np.float32)
    for b in range(B):
        y = (np.asarray(results[2 * b]["out"], dtype=np.float32)
             + np.asarray(results[2 * b + 1]["out"], dtype=np.float32))
        out[b, :, :, 0] = y[0:64].T + b_att[None, :]
        out[b, :, :, 1] = y[64:128].T + b_att[None, :]
    return out


def kernel(Q, K, V, W_att, b_att):
    if "nc" not in _CACHE:
        _CACHE["nc"] = _build_nc()
    nc = _CACHE["nc"]
    in_maps = _prep_inputs(Q, K, V, W_att, b_att)
    res = run_bass_kernel_spmd(nc, in_maps, core_ids=list(range(NCORES)))
    return _gather(res.results, b_att)


# revision 24
# speedup vs baseline: 1.1664x; 1.0115x over previous
"""Trainium2 Bass kernel for nn_AttentionOutput (complex causal leaky-relu attention).

Reference (B=4, N=4096, F=64), per batch:
    sr = (Qr@Kr^T - Qi@Ki^T)/sqrt(N); si = (Qr@Ki^T + Qi@Kr^T)/sqrt(N)
    wr = tril * leaky_relu(sr);        wi = tril * leaky_relu(si)
    out_r = (wr@Vr)@W_att^T + b;       out_i = (wi@Vi)@W_att^T + b

Distribution: 2 cores per batch.  Core parity h processes j-blocks J === h
(mod 2) for ALL 4096 query rows; causal work is then identical across cores
(slot I needs 2I+2 j-blocks), so a single SPMD program serves all 8 cores and
the host sums the two partial outputs per batch.

Host-side layout prep removes every on-device transpose:
  - scores contract over p = f*2+c (128 partitions, ONE matmul per component):
    sr = Qmodr . K^T where Qmodr = Q with odd columns negated, and
    si = Qmodi . K^T where Qmodi = Q with column pairs swapped; K stays plain.
    Both Q variants are fed pre-transposed [128, N].
  - V' = (1/64) V @ W_att^T folds the score scale and the output projection
    into the attention-value matmul (leaky_relu is positively homogeneous).
  - output is stored transposed ([128, N]: y_r^T on rows 0:64, y_i^T on
    64:128); the host untransposes, interleaves, adds bias, sums parities.

v2 structure (vs v1 baseline at 131us):
  - scores for r and i go into ONE 3D PSUM tile [128, 2, 512] (2 banks);
    the relu drain covers both components in ONE DVE/ACT op (fixed-cost
    amortized), alternating VectorE / ScalarE by a 1:2 pattern.
  - y_r^T and y_i^T share ONE [128, 512] PSUM bank: value matmuls are
    col-group packed (r -> partitions 0:64 via stationary cols 0:64,
    i -> partitions 64:128).  The PE runs such M=64 pairs concurrently in
    disjoint column groups, roughly halving value-matmul time.
  - diagonal blocks: the second diag j-block of each slot is fully masked
    for i-offsets < 256 on both core parities, so score matmuls, drains and
    value matmuls subrange to [256:512] there.
  - PSUM: 3 score bufs (6 banks) + 2 y bufs (2 banks) = 8 banks.

leaky_relu lowering (RELU_CORR): leaky(s) = 0.99*relu(s) + 0.01*s.  For
causally-full j-blocks the 0.01*s term telescopes into a per-slot constant
matmul: mcorr = 0.01*sum_J kp_J @ V'_J, precomputed on the host and
accumulated into the y PSUM bank.  Diagonal tiles compute u = mask*s
(VectorE, drains) and w = relu(u), feeding matmuls against 0.01*V' and
0.99*V'.

NOTE: ACT Lrelu reading PSUM hangs TRN2 (empirically) — never emit it.
"""

import numpy as np

import concourse.bacc as bacc
import concourse.tile as tile
from concourse import mybir
from concourse.bass_utils import run_bass_kernel_spmd

B, N, F = 4, 4096, 64
P = 128             # = 2*F: score contraction width / partition count
JB = 128            # j-block width
IBW = 512           # i-block (slot) width
NSLOT = N // IBW    # 8 slots
NJPAR = N // JB // 2  # 16 parity j-blocks per core
NEG = 0.01
SCALE = 1.0 / 64.0  # 1/sqrt(N)
NCORES = 8
DIAG1_OFF = 256     # both parities: 2nd diag block dead for i-offset < 256

_DT = mybir.dt.float32
MM_BF16 = True      # bf16 matmul inputs: half the DMA bytes, 4x fp32 PE rate
SIM_SAFE_LRELU = True  # kept for test.py compat (unused)
_CACHE: dict = {}


def _build_nc():
    nc = bacc.Bacc("TRN2", target_bir_lowering=False, num_devices=NCORES)
    dt = _DT
    mdt = mybir.dt.bfloat16 if MM_BF16 else _DT  # matmul input dtype
    qrT = nc.dram_tensor("qrT", [P, N], mdt, kind="ExternalInput")
    qiT = nc.dram_tensor("qiT", [P, N], mdt, kind="ExternalInput")
    # fused copy of q (dim1 = component) for the late chunks: one DMA
    # descriptor covers both components
    q2f = nc.dram_tensor("q2f", [P, 2, N], mdt, kind="ExternalInput")
    kp = nc.dram_tensor("kp", [P, NJPAR * JB], mdt, kind="ExternalInput")
    # va = 0.99 * V' (relu term), vb = 0.01 * V' (raw term, diagonal only);
    # dim1 = component
    va2 = nc.dram_tensor("va2", [P, 2, NJPAR * F], mdt, kind="ExternalInput")
    vb2 = nc.dram_tensor("vb2", [P, 2, NJPAR * F], mdt, kind="ExternalInput")
    # per-slot correction weights: 0.01 * sum_{full J} kp_J @ V'_J  [P, 64]
    mc2 = nc.dram_tensor("mc2", [P, 2, NSLOT * F], mdt, kind="ExternalInput")
    # one mask per diag block k (broadcast over components on device)
    dmask = nc.dram_tensor("dmask", [JB, 2, IBW], mdt, kind="ExternalInput")
    out = nc.dram_tensor("out", [P, N], mdt, kind="ExternalOutput")

    relu = mybir.ActivationFunctionType.Relu
    mul_op = mybir.AluOpType.mult
    max_op = mybir.AluOpType.max
    add_op = mybir.AluOpType.add

    with tile.TileContext(nc) as tc:
        with (
            tc.tile_pool(name="res", bufs=1) as res,
            tc.tile_pool(name="wp", bufs=3) as wp,
            tc.tile_pool(name="osb", bufs=2) as osb,
            tc.tile_pool(name="spsum", bufs=3, space="PSUM") as spsum,
            tc.tile_pool(name="ypsum", bufs=2, space="PSUM") as ypsum,
        ):
            # ---- input staging; order so slot 0's operands land first ----
            sb_q = res.tile([P, 2, N], mdt, tag="q")
            sb_k = res.tile([P, NJPAR * JB], mdt, tag="k")
            sb_m = res.tile([JB, 2, IBW], mdt, tag="m")
            sb_va = res.tile([P, 2, NJPAR * F], mdt, tag="va")
            sb_vb = res.tile([P, 2, NJPAR * F], mdt, tag="vb")
            sb_mc = res.tile([P, 2, NSLOT * F], mdt, tag="mc")

            # All input DMAs on the Sync queue (out-DMAs go to GpSimd),
            # ordered by first use under the diag-first slot schedule.
            # Early q chunks are split per component for latency; late ones
            # and all v/mcorr tensors are component-fused (1 descriptor).
            nc.sync.dma_start(out=sb_q[:, 0:1, 0:512], in_=qrT[:, 0:512])
            nc.sync.dma_start(out=sb_k[:, 0:256], in_=kp[:, 0:256])
            nc.sync.dma_start(out=sb_q[:, 1:2, 0:512], in_=qiT[:, 0:512])
            nc.sync.dma_start(out=sb_m[:, 0:1, :], in_=dmask[:, 0:1, :])
            nc.sync.dma_start(out=sb_q[:, 0:1, 512:1024], in_=qrT[:, 512:1024])
            nc.sync.dma_start(out=sb_m[:, 1:2, :], in_=dmask[:, 1:2, :])
            nc.sync.dma_start(out=sb_q[:, 1:2, 512:1024], in_=qiT[:, 512:1024])
            nc.sync.dma_start(out=sb_vb[:, :, 0:256], in_=vb2[:, :, 0:256])
            nc.sync.dma_start(out=sb_k[:, 256:512], in_=kp[:, 256:512])
            nc.sync.dma_start(out=sb_va[:, :, 0:256], in_=va2[:, :, 0:256])
            nc.sync.dma_start(out=sb_mc[:, :, 0:192], in_=mc2[:, :, 0:192])
            nc.sync.dma_start(out=sb_k[:, 512:1024], in_=kp[:, 512:1024])
            nc.sync.dma_start(out=sb_q[:, 0:1, 1024:1536], in_=qrT[:, 1024:1536])
            nc.sync.dma_start(out=sb_q[:, 1:2, 1024:1536], in_=qiT[:, 1024:1536])
            nc.sync.dma_start(out=sb_vb[:, :, 256:512], in_=vb2[:, :, 256:512])
            nc.sync.dma_start(out=sb_q[:, 0:1, 1536:2048], in_=qrT[:, 1536:2048])
            nc.sync.dma_start(out=sb_q[:, 1:2, 1536:2048], in_=qiT[:, 1536:2048])
            nc.sync.dma_start(out=sb_va[:, :, 256:512], in_=va2[:, :, 256:512])
            nc.sync.dma_start(out=sb_mc[:, :, 192:512], in_=mc2[:, :, 192:512])
            nc.sync.dma_start(out=sb_k[:, 1024:1536], in_=kp[:, 1024:1536])
            nc.sync.dma_start(out=sb_q[:, :, 2048:2560], in_=q2f[:, :, 2048:2560])
            nc.sync.dma_start(out=sb_vb[:, :, 512:1024], in_=vb2[:, :, 512:1024])
            nc.sync.dma_start(out=sb_q[:, :, 2560:3072], in_=q2f[:, :, 2560:3072])
            nc.sync.dma_start(out=sb_va[:, :, 512:1024], in_=va2[:, :, 512:1024])
            nc.sync.dma_start(out=sb_k[:, 1536:2048], in_=kp[:, 1536:2048])
            nc.sync.dma_start(out=sb_q[:, :, 3072:3584], in_=q2f[:, :, 3072:3584])
            nc.sync.dma_start(out=sb_q[:, :, 3584:4096], in_=q2f[:, :, 3584:4096])
            sb_masks = tuple(
                sb_m[:, k:k + 1, :].broadcast_to([JB, 2, IBW])
                for k in range(2))

            # Warm the PE (HAM clock gate) with dummy matmuls while the first
            # DMAs are in flight, sized to span the DMA head so real matmuls
            # start at 2.4 GHz without being queued behind the warm-up.
            warm_sb = res.tile([P, F], mdt, tag="warm")
            nc.vector.memset(warm_sb[:], 0.0)
            warm_y = ypsum.tile([P, IBW], dt, tag="y")
            for _ in range(18):
                nc.tensor.matmul(warm_y[0:64, 0:64], warm_sb[:], warm_sb[:],
                                 start=True, stop=True)

            drain_ctr = 0  # full-tile relu drains: cycle V,S,S,S
            for s in range(NSLOT):
                cnt = 2 * s + 2
                nfull = cnt - 2
                isl = slice(s * IBW, (s + 1) * IBW)
                y = ypsum.tile([P, IBW], dt, tag="y")

                def sc_pair(p, o=0):
                    """Score matmul pair for j-block p into a fresh PSUM tile."""
                    ksl = slice(p * JB, (p + 1) * JB)
                    qsl = slice(s * IBW + o, (s + 1) * IBW)
                    s2 = spsum.tile([P, 2, IBW], dt, tag="s", name="s2")
                    nc.tensor.matmul(s2[:, 0:1, o:], sb_k[:, ksl],
                                     sb_q[:, 0:1, qsl], start=True, stop=True)
                    nc.tensor.matmul(s2[:, 1:2, o:], sb_k[:, ksl],
                                     sb_q[:, 1:2, qsl], start=True, stop=True)
                    return s2

                def val_pair(p, w, start):
                    vsl = slice(p * F, (p + 1) * F)
                    nc.tensor.matmul(y[0:64, :], sb_va[:, 0:1, vsl],
                                     w[:, 0:1, :], start=start, stop=False)
                    nc.tensor.matmul(y[64:128, :], sb_va[:, 1:2, vsl],
                                     w[:, 1:2, :], start=start, stop=False)

                # Full blocks, software-pipelined two deep: the value pair
                # for block p is emitted after block p+2's scores so its
                # drain has cover (the tensor queue is strict FIFO).
                pend = []  # (p, w) awaiting value pairs
                for p in range(nfull):
                    s2 = sc_pair(p)
                    w = wp.tile([P, 2, IBW], mdt, tag="w")
                    # Small slots drain on ScalarE only (VectorE is busy with
                    # their diag chains); big slots give 1/3 to VectorE.
                    on_v = (s >= 5 and p % 5 in (1, 3) and p <= nfull - 3) \
                        or (s == 4 and p == 1)
                    if on_v:
                        nc.vector.tensor_scalar_max(w[:], s2[:], 0.0)
                    else:
                        nc.scalar.activation(w[:], s2[:], relu)
                    if len(pend) == 3:
                        q0 = pend.pop(0)
                        val_pair(q0[0], q0[1], q0[0] == 0)
                    pend.append((p, w))

                # Diagonal blocks: u = mask*s (drains), w2 = max(100u, u) =
                # u + 99*relu(u), so vb @ w2 = 0.01*V'*u + 0.99*V'*relu(u).
                w2s = []
                for k in range(2):
                    p = nfull + k
                    o = DIAG1_OFF if k == 1 else 0
                    s2 = sc_pair(p, o)
                    mk = sb_masks[k]
                    u = wp.tile([P, 2, IBW], mdt, tag="u", bufs=2)
                    nc.vector.tensor_tensor(out=u[:, :, o:], in0=s2[:, :, o:],
                                            in1=mk[:, :, o:], op=mul_op)
                    w2 = wp.tile([P, 2, IBW], mdt, tag="w2", bufs=4)
                    nc.vector.scalar_tensor_tensor(
                        out=w2[:, :, o:], in0=u[:, :, o:], scalar=100.0,
                        in1=u[:, :, o:], op0=mul_op, op1=max_op)
                    w2s.append((w2, o, p))
                    if pend:
                        q0 = pend.pop(0)
                        val_pair(q0[0], q0[1], q0[0] == 0)

                for q0 in pend:
                    val_pair(q0[0], q0[1], q0[0] == 0)
                pend = []

                # Correction pair (no drain dependency) covers the diag
                # chains' latency: y += (0.01 * sum_full kp_J @ V'_J)^T @ q
                if s > 0:
                    msl = slice(s * F, (s + 1) * F)
                    nc.tensor.matmul(y[0:64, :], sb_mc[:, 0:1, msl],
                                     sb_q[:, 0:1, isl], start=False, stop=False)
                    nc.tensor.matmul(y[64:128, :], sb_mc[:, 1:2, msl],
                                     sb_q[:, 1:2, isl], start=False, stop=False)

                # Diagonal vb pairs close the slot.
                for k, (w2, o, p) in enumerate(w2s):
                    vsl = slice(p * F, (p + 1) * F)
                    first = s == 0 and k == 0
                    last = k == 1
                    nc.tensor.matmul(y[0:64, o:], sb_vb[:, 0:1, vsl],
                                     w2[:, 0:1, o:], start=first, stop=last)
                    nc.tensor.matmul(y[64:128, o:], sb_vb[:, 1:2, vsl],
                                     w2[:, 1:2, o:], start=first, stop=last)

                # tail: accumulator to SBUF bf16 (alternate V/S), DMA out;
                # the final slot splits copy and DMA so its exposed tail is
                # as short as possible
                y_sb = osb.tile([P, IBW], mdt, tag="ysb")
                if s == NSLOT - 1:
                    nc.vector.tensor_copy(y_sb[0:64, :], y[0:64, :])
                    nc.scalar.copy(y_sb[64:128, :], y[64:128, :])
                    nc.gpsimd.dma_start(out=out[0:32, isl], in_=y_sb[0:32, :])
                    nc.sync.dma_start(out=out[32:64, isl], in_=y_sb[32:64, :])
                    nc.gpsimd.dma_start(out=out[64:96, isl], in_=y_sb[64:96, :])
                    nc.sync.dma_start(out=out[96:128, isl], in_=y_sb[96:128, :])
                elif s == NSLOT - 2:
                    nc.scalar.copy(y_sb[:], y[:])
                    nc.gpsimd.dma_start(out=out[0:64, isl], in_=y_sb[0:64, :])
                    nc.sync.dma_start(out=out[64:128, isl], in_=y_sb[64:128, :])
                else:
                    if s % 2 == 0:
                        nc.vector.tensor_copy(y_sb[:], y[:])
                    else:
                        nc.scalar.copy(y_sb[:], y[:])
                    nc.gpsimd.dma_start(out=out[:, isl], in_=y_sb[:])
    nc.compile()
    return nc


def _prep_inputs(Q, K, V, W_att, b_att):
    """Host-side re-layout: per-core in_maps for run_bass_kernel_spmd."""
    Q = np.asarray(Q, dtype=np.float32)
    K = np.asarray(K, dtype=np.float32)
    V = np.asarray(V, dtype=np.float32)
    W_att = np.asarray(W_att, dtype=np.float32)

    Qf = Q.reshape(B, N, P)          # [b, i, f*2+c]
    Kf = K.reshape(B, N, P)
    Vpr = SCALE * (V[..., 0] @ W_att.T)   # [B, N, F]
    Vpi = SCALE * (V[..., 1] @ W_att.T)

    # causal masks for a slot's last two parity j-blocks, per core parity h:
    # diagonal sub-block d = 2k+h of the slot's group of 4; duplicated along
    # a component axis -> [2, JB, 2, IBW]
    jj = np.arange(JB)[:, None]
    ii = np.arange(IBW)[None, :]
    masks = {}
    for h in (0, 1):
        masks[h] = np.stack([(ii >= jj + JB * (2 * k + h)).astype(np.float32)
                             for k in range(2)], axis=1)   # [JB, 2, IBW]

    if MM_BF16:
        import ml_dtypes
        cvt = lambda a: np.ascontiguousarray(a).astype(ml_dtypes.bfloat16)
    else:
        cvt = lambda a: np.ascontiguousarray(a, dtype=np.float32)

    in_maps = []
    for c in range(NCORES):
        b, h = divmod(c, 2)
        Qmodr = Qf[b].copy()
        Qmodr[:, 1::2] *= -1.0
        Qmodi = np.empty_like(Qf[b])
        Qmodi[:, 0::2] = Qf[b][:, 1::2]
        Qmodi[:, 1::2] = Qf[b][:, 0::2]
        # parity-packed K: [P, NJPAR*JB], position pp holds block J = 2*pp+h
        kp3 = Kf[b].reshape(N // JB, JB, P)[h::2]          # [16, j, p]
        kp = kp3.transpose(2, 0, 1).reshape(P, -1)         # [p, pp*JB+j]
        vr3 = Vpr[b].reshape(N // JB, JB, F)[h::2]         # [16, j, f]
        vi3 = Vpi[b].reshape(N // JB, JB, F)[h::2]
        vpr = vr3.transpose(1, 0, 2).reshape(JB, -1)       # [j, pp*F+f]
        vpi = vi3.transpose(1, 0, 2).reshape(JB, -1)
        # per-slot correction: 0.01 * sum over FULL blocks (pos < cnt-2 = 2s)
        prod_r = np.einsum('bjp,bjf->bpf', kp3, vr3)       # [16, p, f]
        prod_i = np.einsum('bjp,bjf->bpf', kp3, vi3)
        pre_r = np.concatenate(
            [np.zeros((1, P, F), np.float32), np.cumsum(prod_r, axis=0)])
        pre_i = np.concatenate(
            [np.zeros((1, P, F), np.float32), np.cumsum(prod_i, axis=0)])
        mcr = np.concatenate([NEG * pre_r[2 * s] for s in range(NSLOT)], axis=1)
        mci = np.concatenate([NEG * pre_i[2 * s] for s in range(NSLOT)], axis=1)
        qrt, qit = Qmodr.T, Qmodi.T
        in_maps.append({
            "qrT": cvt(qrt),
            "qiT": cvt(qit),
            "q2f": cvt(np.stack([qrt, qit], axis=1)),
            "kp": cvt(kp),
            "va2": cvt((1.0 - NEG) * np.stack([vpr, vpi], axis=1)),
            "vb2": cvt(NEG * np.stack([vpr, vpi], axis=1)),
            "mc2": cvt(np.stack([mcr, mci], axis=1)),
            "dmask": cvt(masks[h]),
        })
    return in_maps


def _gather(results, b_att):
    b_att = np.asarray(b_att, dtype=np.float32)
    out = np.empty((B, N, F, 2), dtype=np.float32)
    for b in range(B):
        y = (np.asarray(results[2 * b]["out"], dtype=np.float32)
             + np.asarray(results[2 * b + 1]["out"], dtype=np.float32))
        out[b, :, :, 0] = y[0:64].T + b_att[None, :]
        out[b, :, :, 1] = y[64:128].T + b_att[None, :]
    return out


def kernel(Q, K, V, W_att, b_att):
    if "nc" not in _CACHE:
        _CACHE["nc"] = _build_nc()
    nc = _CACHE["nc"]
    in_maps = _prep_inputs(Q, K, V, W_att, b_att)
    res = run_bass_kernel_spmd(nc, in_maps, core_ids=list(range(NCORES)))
    return _gather(res.results, b_att)
